# revision 1
# baseline (speedup 1.0000x reference)
"""Trainium2 Bass kernel for nn_DecoderBlock (self-attn + cross-attn + FFN, post-LN).

Sharding (8 cores = 2 batch groups x 4 cores):
 - Attention is head-parallel within each group (4 heads/core, full S), which
   keeps the causal-mask instruction stream rank-uniform (SPMD).
 - Everything else (out-projections, LayerNorms, FFN) is sequence-parallel:
   each core owns a 512-token strip and uses the FULL Wo/W1/W2 weights, so
   there are no partial sums and no AllReduces.
 - All resharding transitions (attention-out head->strip, cross-Q
   strip->head) are 8-core AllToAlls of ~2MB, combined on the receiver with
   a per-core 0/1 mask (SPMD keeps all addressing rank-uniform; rank
   variation lives entirely in host-supplied data).
 - All matmul operands are bf16 (fp32 PSUM accumulation); the residual/LN
   stream is kept fp32.
"""

import os
import sys

sys.path.insert(0, "/opt/trn_rl_repo")

from contextlib import ExitStack

import ml_dtypes
import numpy as np

import concourse.bacc as bacc
import concourse.tile as tile
from concourse import mybir
from concourse.bass_utils import run_bass_kernel_spmd

F32R = mybir.dt.float32r
F32 = mybir.dt.float32
BF16 = mybir.dt.bfloat16
AF = mybir.ActivationFunctionType
ALU = mybir.AluOpType

B = 2
D = 1024
H = 16
HD = 64
FF = 4 * D
NCORES = 8
G = 4                  # cores per batch group
HL = H // G            # 4 local heads
DC = HL * HD           # 256 local q/k/v features
DCA = HL * (HD + 1)    # 260: V augmented with a ones column per head
SW = 512               # tokens per core strip
CW = 512               # free-dim chunk width
DT = D // 128          # 8 feature partition-tiles
FT = FF // 128         # 32 ffn partition-tiles
GROUPS4 = [[0, 1, 2, 3], [4, 5, 6, 7]]
GROUP8 = [[0, 1, 2, 3, 4, 5, 6, 7]]

_nc_cache = {}


def _build(S, M, causal):
    nc = bacc.Bacc(None, target_bir_lowering=False, num_devices=NCORES)
    NCH = S // CW          # 4 query chunks (full S)
    MT = M // 128          # 16 key tiles (cross)
    ST = S // 128          # 16 key tiles (self)

    dp = nc.declare_dram_parameter
    xT = dp("xT", [D, S], BF16, isOutput=False)
    xs = dp("xs", [D, SW], F32R, isOutput=False)
    memT = dp("memT", [D, M], BF16, isOutput=False)
    wq = dp("wq", [D, DC], BF16, isOutput=False)
    wk = dp("wk", [D, DC], BF16, isOutput=False)
    wv = dp("wv", [D, DCA], BF16, isOutput=False)
    bq = dp("bq", [DC, 1], F32, isOutput=False)
    bk = dp("bk", [DC, 1], F32, isOutput=False)
    bva = dp("bva", [1, DCA], F32, isOutput=False)
    wo = dp("wo", [D, D], BF16, isOutput=False)
    bo = dp("bo", [D, 1], F32, isOutput=False)
    wqx = dp("wqx", [D, D], BF16, isOutput=False)
    wkx = dp("wkx", [D, DC], BF16, isOutput=False)
    wvx = dp("wvx", [D, DCA], BF16, isOutput=False)
    bqx = dp("bqx", [D, 1], F32, isOutput=False)
    bkx = dp("bkx", [DC, 1], F32, isOutput=False)
    bvxa = dp("bvxa", [1, DCA], F32, isOutput=False)
    wox = dp("wox", [D, D], BF16, isOutput=False)
    box = dp("box", [D, 1], F32, isOutput=False)
    w1 = dp("w1", [D, FF], BF16, isOutput=False)
    b1 = dp("b1", [FF, 1], F32, isOutput=False)
    w2 = dp("w2", [FF, D], BF16, isOutput=False)
    b2 = dp("b2", [D, 1], F32, isOutput=False)
    lng = dp("lng", [3 * D, 1], F32, isOutput=False)
    lnb = dp("lnb", [3 * D, 1], F32, isOutput=False)
    ones = dp("ones", [128, 128], F32R, isOutput=False)
    gmask = dp("gmask", [128, 2], F32, isOutput=False)
    gmask4 = dp("gmask4", [128, 4], F32, isOutput=False)
    outT = dp("outT", [D, SW], F32R, isOutput=True)

    with tile.TileContext(nc) as tc, ExitStack() as st:
        ep = st.enter_context
        constp = ep(tc.tile_pool(name="const", bufs=1))
        dramp = ep(tc.tile_pool(name="dram", bufs=1, space="DRAM"))

        DMA_ENG = [nc.sync, nc.gpsimd, nc.scalar]
        # gpsimd's queue sits behind the collectives-prelude barrier; route
        # startup loads to the HWDGE engines only.
        LOAD_ENG = [nc.sync, nc.scalar]

        # ---- DRAM bounce buffers for collectives ----
        t1in = dramp.tile([2 * D, SW], BF16, name="t1in")
        t1out = dramp.tile([2 * D, SW], BF16, name="t1out")
        q2in = dramp.tile([2 * D, SW], BF16, name="q2in")
        q2out = dramp.tile([2 * D, SW], BF16, name="q2out")
        t3in = dramp.tile([2 * D, SW], BF16, name="t3in")
        t3out = dramp.tile([2 * D, SW], BF16, name="t3out")
        t3in2 = dramp.tile([2 * D, SW], BF16, name="t3in2")
        t3out2 = dramp.tile([2 * D, SW], BF16, name="t3out2")

        # ---- constants ----
        ones_t = constp.tile([128, 128], F32R, name="ones_t")
        nc.gpsimd.dma_start(out=ones_t[:], in_=ones[:, :])
        eps_t = constp.tile([128, 1], F32, name="eps_t")
        nc.vector.memset(eps_t[:], 1e-5)
        gmask_t = constp.tile([128, 2], F32, name="gmask_t")
        nc.gpsimd.dma_start(out=gmask_t[:], in_=gmask[:, :])
        gmask4_t = constp.tile([128, 4], F32, name="gmask4_t")
        nc.gpsimd.dma_start(out=gmask4_t[:], in_=gmask4[:, :])

        def bias_tiles(src, n, prefix):
            ts = []
            for i in range(n):
                t = constp.tile([128, 1], F32, name=f"{prefix}{i}")
                nc.gpsimd.dma_start(out=t[:], in_=src[i * 128:(i + 1) * 128, :])
                ts.append(t)
            return ts

        bq_t = bias_tiles(bq, 2, "bq")
        bk_t = bias_tiles(bk, 2, "bk")
        bqx_t = bias_tiles(bqx, DT, "bqx")
        bkx_t = bias_tiles(bkx, 2, "bkx")
        bo_t = bias_tiles(bo, DT, "bo")
        box_t = bias_tiles(box, DT, "box")
        b1_t = bias_tiles(b1, FT, "b1")
        b2_t = bias_tiles(b2, DT, "b2")
        lng_t = bias_tiles(lng, 3 * DT, "lng")
        lnb_t = bias_tiles(lnb, 3 * DT, "lnb")
        bva_t = constp.tile([128, DCA], F32, name="bva_t")
        nc.gpsimd.dma_start(out=bva_t[:], in_=bva[:, :].to_broadcast([128, DCA]))
        bvxa_t = constp.tile([128, DCA], F32, name="bvxa_t")
        nc.gpsimd.dma_start(out=bvxa_t[:],
                            in_=bvxa[:, :].to_broadcast([128, DCA]))

        # ================= helpers =================
        _wload_rr = [0]

        def load_w_small(pool, src, cols, tag):
            """Whole [DT*128, cols] weight in ONE DMA -> [128, DT*cols] tile;
            returns per-d [128, cols] slice APs."""
            t = pool.tile([128, DT * cols], BF16, name=tag, tag=tag, bufs=1)
            eng = LOAD_ENG[_wload_rr[0] % len(LOAD_ENG)]
            _wload_rr[0] += 1
            eng.dma_start(
                out=t[:].rearrange("p (d j) -> p d j", j=cols),
                in_=src[:, :].rearrange("(d p) j -> p d j", p=128))
            return [t[:, d * cols:(d + 1) * cols] for d in range(DT)]

        def load_w_rows(pool, src, cols, tag, nrows):
            """[128, cols] row-tiles of a big weight (rows = contraction)."""
            ts = []
            for k in range(nrows):
                t = pool.tile([128, cols], BF16, name=tag, tag=tag, bufs=nrows)
                DMA_ENG[k % len(DMA_ENG)].dma_start(
                    out=t[:], in_=src[k * 128:(k + 1) * 128, :])
                ts.append(t)
            return ts

        def project_qk(qt_pair, w_tiles, b_tiles, src_tiles, psum, tag,
                       interleave=None):
            """q/k projection: 2x [128, S] packed tiles (2 heads each)."""
            it = iter(interleave) if interleave is not None else None
            for sc in range(NCH):
                sl = slice(sc * CW, (sc + 1) * CW)
                for t in range(2):
                    ps = psum.tile([128, CW], F32, name=f"{tag}ps", tag="qkps",
                                   bufs=2)
                    for d in range(DT):
                        nc.tensor.matmul(
                            out=ps[:],
                            lhsT=w_tiles[d][:, t * 128:(t + 1) * 128],
                            rhs=src_tiles[d][:, sl],
                            start=(d == 0), stop=(d == DT - 1),
                        )
                    nc.scalar.activation(out=qt_pair[t][:, sl], in_=ps[:],
                                         func=AF.Identity, bias=b_tiles[t][:],
                                         scale=1.0)
                if it is not None:
                    next(it, None)

        def project_v(vpool, w_tiles, bias_bc, src_tiles, psum, n_tok, tag,
                      interleave=None):
            """v projection: n_tok/128 tiles of [128 tok, DCA]."""
            it = iter(interleave) if interleave is not None else None
            vs = []
            for s_t in range(n_tok // 128):
                ps = psum.tile([128, DCA], F32, name=f"{tag}ps", tag="vps",
                               bufs=2)
                for d in range(DT):
                    nc.tensor.matmul(
                        out=ps[:],
                        lhsT=src_tiles[d][:, s_t * 128:(s_t + 1) * 128],
                        rhs=w_tiles[d][:],
                        start=(d == 0), stop=(d == DT - 1))
                vt = vpool.tile([128, DCA], BF16, name=f"{tag}v", tag="vs",
                                bufs=2 * ST)
                nc.vector.tensor_tensor(out=vt[:], in0=ps[:], in1=bias_bc[:],
                                        op=ALU.add)
                vs.append(vt)
                if it is not None and s_t % 2 == 1:
                    next(it, None)
            return vs

        def attention(apool, ppool, dpool, q_tiles, k_tiles, v_tiles, n_keys,
                      use_mask, psc, ppv, tag, chunk_cb=None):
            """Full-S head-sharded attention; returns 2 packed [128, S]
            bf16 tiles (2 heads each). The two heads of a pair run their
            score matmuls on disjoint PE row-groups (concurrent)."""
            a_packed = [apool.tile([128, S], BF16, name=f"{tag}{t}", tag="attn",
                                   bufs=2) for t in range(2)]
            kt_total = n_keys // 128
            kpc = CW // 128
            for qc in range(NCH):
                sl = slice(qc * CW, (qc + 1) * CW)
                for ti in range(HL // 2):
                    kts = range(min(kt_total, kpc * (qc + 1)) if use_mask
                                else kt_total)
                    n_kt = len(kts)
                    pv_ps = [ppv.tile([65, CW], F32, name=f"{tag}pv{par}",
                                      tag="pvps", bufs=4) for par in range(2)]

                    def emit_scores(kt):
                        s_ps = [psc.tile([128, CW], F32, name=f"{tag}s{par}",
                                         tag="scps", bufs=4)
                                for par in range(2)]
                        for par in range(2):
                            nc.tensor.matmul(
                                out=s_ps[par][:],
                                lhsT=k_tiles[ti][par * 64:(par + 1) * 64,
                                                 kt * 128:(kt + 1) * 128],
                                rhs=q_tiles[ti][par * 64:(par + 1) * 64, sl],
                                start=True, stop=True,
                                tile_position=(64 * par, 0),
                            )
                        return s_ps

                    def emit_pv(s_ps, kt, i):
                        for par in range(2):
                            h = 2 * ti + par
                            p_t = ppool.tile([128, CW], BF16, name=f"{tag}p",
                                             tag="p", bufs=6)
                            nc.scalar.activation(out=p_t[:], in_=s_ps[par][:],
                                                 func=AF.Exp)
                            if use_mask and kt >= kpc * qc:
                                p_m = ppool.tile([128, CW], BF16,
                                                 name=f"{tag}pm", tag="p",
                                                 bufs=6)
                                nc.gpsimd.affine_select(
                                    out=p_m[:], in_=p_t[:], pattern=[[1, CW]],
                                    compare_op=ALU.is_ge, fill=0.0,
                                    base=qc * CW - kt * 128,
                                    channel_multiplier=-1)
                                p_use = p_m
                            else:
                                p_use = p_t
                            nc.tensor.matmul(
                                out=pv_ps[par][:],
                                lhsT=v_tiles[kt][:, h * 65:(h + 1) * 65],
                                rhs=p_use[:],
                                start=(i == 0), stop=(i == n_kt - 1),
                            )

                    # one-step lookahead: scores of kt+1 are emitted before
                    # the exp-gated PV of kt so the PE never queues behind
                    # a semaphore wait on the scalar engine
                    prev = None
                    for i, kt in enumerate(kts):
                        s_ps = emit_scores(kt)
                        if prev is not None:
                            emit_pv(*prev)
                        prev = (s_ps, kt, i)
                    emit_pv(*prev)
                    for par in range(2):
                        # softmax denominator: reciprocal of row 64, broadcast
                        drc = dpool.tile([65, CW], F32, name=f"{tag}drc",
                                         tag="drc", bufs=2)
                        nc.vector.reciprocal(out=drc[64:65, :],
                                             in_=pv_ps[par][64:65, :])
                        dn0 = dpool.tile([1, CW], F32, name=f"{tag}dn0",
                                         tag="dn0", bufs=2)
                        nc.sync.dma_start(out=dn0[0:1, :], in_=drc[64:65, :])
                        db = dpool.tile([64, CW], F32, name=f"{tag}db",
                                        tag="db", bufs=2)
                        nc.gpsimd.partition_broadcast(db[:], dn0[0:1, :])
                        nc.vector.tensor_tensor(
                            out=a_packed[ti][par * 64:(par + 1) * 64, sl],
                            in0=pv_ps[par][0:64, :], in1=db[:], op=ALU.mult)
            return a_packed

        def a2a_send(a_packed, zin, zout):
            """Spill the packed attention output and kick off the AllToAll."""
            for j in range(2 * G):
                for ti in range(2):
                    DMA_ENG[(2 * j + ti) % len(DMA_ENG)].dma_start(
                        out=zin[j * 256 + ti * 128: j * 256 + (ti + 1) * 128, :],
                        in_=a_packed[ti][:, (j % G) * SW:(j % G + 1) * SW])
            nc.gpsimd.collective_compute(
                "AllToAll", ALU.bypass, replica_groups=GROUP8,
                ins=[zin.opt()], outs=[zout.opt()])

        def a2a_send_half(a_packed, zin, zout, base_strip):
            """Spill strips [base_strip, base_strip+2) and AllToAll them."""
            for j in range(2 * G):
                for ti in range(2):
                    s0 = (base_strip + (j % 2)) * SW
                    DMA_ENG[(2 * j + ti) % len(DMA_ENG)].dma_start(
                        out=zin[j * 256 + ti * 128: j * 256 + (ti + 1) * 128, :],
                        in_=a_packed[ti][:, s0:s0 + SW])
            nc.gpsimd.collective_compute(
                "AllToAll", ALU.bypass, replica_groups=GROUP8,
                ins=[zin.opt()], outs=[zout.opt()])

        def a2a_recv4(zout_a, zout_b, tmp_pool, a_pool, tag):
            """Strip assembly from the two half AllToAlls (4-way mask)."""
            a_str = []
            for ft in range(DT):
                srcs = [(zout_a, 0), (zout_a, D), (zout_b, 0), (zout_b, D)]
                loads = []
                for idx, (z, off) in enumerate(srcs):
                    t = tmp_pool.tile([128, SW], BF16, name=f"{tag}r{idx}",
                                      tag="atmp", bufs=8)
                    DMA_ENG[(ft + idx) % len(DMA_ENG)].dma_start(
                        out=t[:], in_=z[off + ft * 128: off + (ft + 1) * 128, :])
                    loads.append(t)
                a = a_pool.tile([128, SW], BF16, name=f"{tag}a", tag="astr",
                                bufs=DT)
                nc.vector.tensor_scalar_mul(out=a[:], in0=loads[0][:],
                                            scalar1=gmask4_t[:, 0:1])
                for idx in range(1, 4):
                    nc.vector.scalar_tensor_tensor(
                        out=a[:], in0=loads[idx][:],
                        scalar=gmask4_t[:, idx:idx + 1], in1=a[:],
                        op0=ALU.mult, op1=ALU.add)
                a_str.append(a)
            return a_str

        def a2a_recv(zout, tmp_pool, a_pool, tag):
            """Read back my strip: masked add of the two group blocks."""
            a_str = []
            for ft in range(DT):
                top = tmp_pool.tile([128, SW], BF16, name=f"{tag}t", tag="atmp",
                                    bufs=8)
                bot = tmp_pool.tile([128, SW], BF16, name=f"{tag}b", tag="atmp",
                                    bufs=8)
                DMA_ENG[ft % len(DMA_ENG)].dma_start(
                    out=top[:], in_=zout[ft * 128:(ft + 1) * 128, :])
                DMA_ENG[(ft + 1) % len(DMA_ENG)].dma_start(
                    out=bot[:], in_=zout[D + ft * 128:D + (ft + 1) * 128, :])
                a = a_pool.tile([128, SW], BF16, name=f"{tag}a", tag="astr",
                                bufs=DT)
                nc.vector.tensor_scalar_mul(out=a[:], in0=top[:],
                                            scalar1=gmask_t[:, 0:1])
                nc.vector.scalar_tensor_tensor(
                    out=a[:], in0=bot[:], scalar=gmask_t[:, 1:2], in1=a[:],
                    op0=ALU.mult, op1=ALU.add)
                a_str.append(a)
            return a_str

        def out_project_strip(wo_tiles, b_tiles, a_str, z32, psum, tag):
            """z32[d] = wo.T @ a_str + bias + z32  (residual add in place)."""
            for d in range(DT):
                ps = psum.tile([128, CW], F32, name=f"{tag}ps", tag="ops",
                               bufs=2)
                for kt in range(DT):
                    nc.tensor.matmul(
                        out=ps[:], lhsT=wo_tiles[kt][:, d * 128:(d + 1) * 128],
                        rhs=a_str[kt][:],
                        start=(kt == 0), stop=(kt == DT - 1))
                nc.vector.scalar_tensor_tensor(
                    out=z32[d][:], in0=ps[:], scalar=b_tiles[d][:],
                    in1=z32[d][:], op0=ALU.add, op1=ALU.add)

        def layer_norm_strip(z32, z16, ln_idx, psum, sqpool, tmppool, tag):
            """Post-LN on the [D, SW] fp32 strip; writes bf16 copy z16."""
            mps = psum.tile([128, CW], F32, name=f"{tag}m", tag="lnps", bufs=2)
            for d in range(DT):
                nc.tensor.matmul(out=mps[:], lhsT=ones_t[:],
                                 rhs=z32[d][:],
                                 start=(d == 0), stop=(d == DT - 1))
            mu = tmppool.tile([128, CW], F32, name=f"{tag}mu", tag="mu", bufs=1)
            nc.vector.tensor_copy(out=mu[:], in_=mps[:])
            qps = psum.tile([128, CW], F32, name=f"{tag}q", tag="lnps", bufs=2)
            for d in range(DT):
                sq = sqpool.tile([128, CW], F32R, name=f"{tag}sq", tag="sq",
                                 bufs=2)
                nc.scalar.activation(out=sq[:], in_=z32[d][:], func=AF.Square)
                nc.tensor.matmul(out=qps[:], lhsT=ones_t[:], rhs=sq[:],
                                 start=(d == 0), stop=(d == DT - 1))
            var = tmppool.tile([128, CW], F32, name=f"{tag}v", tag="tv", bufs=1)
            nc.vector.tensor_tensor(out=var[:], in0=mu[:], in1=mu[:],
                                    op=ALU.mult)
            nc.vector.tensor_tensor(out=var[:], in0=qps[:], in1=var[:],
                                    op=ALU.subtract)
            std = tmppool.tile([128, CW], F32, name=f"{tag}sd", tag="std",
                               bufs=1)
            nc.scalar.activation(out=std[:], in_=var[:], func=AF.Sqrt,
                                 bias=eps_t[:], scale=1.0)
            rstd = tmppool.tile([128, CW], F32, name=f"{tag}r", tag="rstd",
                                bufs=1)
            nc.vector.reciprocal(out=rstd[:], in_=std[:])
            for d in range(DT):
                xm = tmppool.tile([128, CW], F32, name=f"{tag}x", tag="xm",
                                  bufs=2)
                nc.vector.tensor_tensor(out=xm[:], in0=z32[d][:], in1=mu[:],
                                        op=ALU.subtract)
                nc.vector.tensor_tensor(out=xm[:], in0=xm[:], in1=rstd[:],
                                        op=ALU.mult)
                nc.vector.tensor_scalar(
                    out=z32[d][:], in0=xm[:],
                    scalar1=lng_t[ln_idx * DT + d][:],
                    scalar2=lnb_t[ln_idx * DT + d][:],
                    op0=ALU.mult, op1=ALU.add)
                if z16 is not None:
                    nc.scalar.activation(out=z16[d][:], in_=z32[d][:],
                                         func=AF.Identity)

        # ================= pipeline =================
        # Long-lived pools first (strict LIFO pool order is required).
        zp = ep(tc.tile_pool(name="zp", bufs=DT))
        z16p = ep(tc.tile_pool(name="z16p", bufs=DT))
        wop = ep(tc.tile_pool(name="wop", bufs=DT))
        apool = ep(tc.tile_pool(name="apool", bufs=2))
        atmpp = ep(tc.tile_pool(name="atmpp", bufs=8))
        astrp = ep(tc.tile_pool(name="astrp", bufs=DT))

        attn_stack = ExitStack()
        ap2 = attn_stack.enter_context
        mpool = ap2(tc.tile_pool(name="memp", bufs=DT))
        qkp = ap2(tc.tile_pool(name="qk", bufs=8))
        vp = ap2(tc.tile_pool(name="vp", bufs=2 * ST))
        wqkp = ap2(tc.tile_pool(name="wqk", bufs=6 * DT))
        wvp = ap2(tc.tile_pool(name="wvp", bufs=2 * DT))

        # ---- P0/P1: loads + self QKV ----
        x_fm = []
        with tc.tile_pool(name="xp", bufs=DT) as xpool:
            x_fm = [xpool.tile([128, S], BF16, name="xfm", tag="x", bufs=DT)
                    for _ in range(DT)]

            def load_x_chunk(sc):
                for d in range(DT):
                    LOAD_ENG[d % len(LOAD_ENG)].dma_start(
                        out=x_fm[d][:, sc * CW:(sc + 1) * CW],
                        in_=xT[d * 128:(d + 1) * 128, sc * CW:(sc + 1) * CW])

            # x chunk 0 first, then the self-QKV weights (so the first
            # projection matmul is not gated behind the full 4MB of x),
            # then the remaining chunks
            load_x_chunk(0)

            z16 = [z16p.tile([128, SW], BF16, name="z16", tag="z16", bufs=DT)
                   for _ in range(DT)]

            wq_t = load_w_small(wqkp, wq, DC, "wqt")
            wk_t = load_w_small(wqkp, wk, DC, "wkt")
            wv_t = load_w_small(wvp, wv, DCA, "wvt")
            for sc in range(1, NCH):
                load_x_chunk(sc)
            wkx_t = load_w_small(wqkp, wkx, DC, "wkxt")
            wvx_t = load_w_small(wvp, wvx, DCA, "wvxt")

            q_s = [qkp.tile([128, S], BF16, name="qs", tag="qk", bufs=8)
                   for _ in range(2)]
            k_s = [qkp.tile([128, S], BF16, name="ks", tag="qk", bufs=8)
                   for _ in range(2)]
            with tc.tile_pool(name="ps1", bufs=2, space="PSUM") as ps1, \
                 tc.tile_pool(name="ps1v", bufs=2, space="PSUM") as ps1v:
                project_qk(q_s, wq_t, bq_t, x_fm, ps1, "qs")
                project_qk(k_s, wk_t, bk_t, x_fm, ps1, "ks")
                v_s = project_v(vp, wv_t, bva_t, x_fm, ps1v, S, "vs")

            z32 = []
            for d in range(DT):
                t = zp.tile([128, SW], F32R, name="z32", tag="z32", bufs=DT)
                LOAD_ENG[d % len(LOAD_ENG)].dma_start(
                    out=t[:], in_=xs[d * 128:(d + 1) * 128, :])
                z32.append(t)
            m_fm = []
            for d in range(DT):
                t = mpool.tile([128, M], BF16, name="mfm", tag="m", bufs=DT)
                LOAD_ENG[d % len(LOAD_ENG)].dma_start(
                    out=t[:], in_=memT[d * 128:(d + 1) * 128, :])
                m_fm.append(t)

        # x_fm freed here (pool closed); z32 holds the residual strip.

        # ---- P2/P3: self attention; cross K/V fills collective windows ----
        kx = [qkp.tile([128, M], BF16, name="kx", tag="qk", bufs=8)
              for _ in range(2)]
        vx = []

        def ckv_steps(psx, psxv, lo, hi):
            """Emit cross-K/V projection steps [lo, hi). Steps 0..3 are kx
            512-chunks; steps 4..11 are vx token-tile pairs."""
            for step in range(lo, hi):
                if step < 4:
                    sc = step
                    sl = slice(sc * CW, (sc + 1) * CW)
                    for t in range(2):
                        ps = psx.tile([128, CW], F32, name="kxps", tag="kxps",
                                      bufs=2)
                        for d in range(DT):
                            nc.tensor.matmul(
                                out=ps[:],
                                lhsT=wkx_t[d][:, t * 128:(t + 1) * 128],
                                rhs=m_fm[d][:, sl],
                                start=(d == 0), stop=(d == DT - 1))
                        nc.vector.tensor_scalar_add(out=kx[t][:, sl],
                                                    in0=ps[:],
                                                    scalar1=bkx_t[t][:])
                else:
                    g2 = step - 4
                    for s_t in (2 * g2, 2 * g2 + 1):
                        ps = psxv.tile([128, DCA], F32, name="vxps", tag="vxps",
                                       bufs=2)
                        for d in range(DT):
                            nc.tensor.matmul(
                                out=ps[:],
                                lhsT=m_fm[d][:, s_t * 128:(s_t + 1) * 128],
                                rhs=wvx_t[d][:],
                                start=(d == 0), stop=(d == DT - 1))
                        vt = vp.tile([128, DCA], BF16, name="vx", tag="vs",
                                     bufs=2 * ST)
                        nc.vector.tensor_tensor(out=vt[:], in0=ps[:],
                                                in1=bvxa_t[:], op=ALU.add)
                        vx.append(vt)

        with tc.tile_pool(name="pp1", bufs=7) as pp1, \
             tc.tile_pool(name="dn1", bufs=6) as dn1, \
             tc.tile_pool(name="ps2s", bufs=4, space="PSUM") as ps2s, \
             tc.tile_pool(name="ps2v", bufs=4, space="PSUM") as ps2v:
            a_sa = attention(apool, pp1, dn1, q_s, k_s, v_s, S, causal,
                             ps2s, ps2v, "sa")

        # ---- P4/P5: A2A self (cross-KV runs during it), out-proj, LN1, AG ----
        a2a_send(a_sa, t1in, t1out)
        with tc.tile_pool(name="psx", bufs=2, space="PSUM") as psx, \
             tc.tile_pool(name="psxv", bufs=2, space="PSUM") as psxv:
            ckv_steps(psx, psxv, 0, 6)
        wo_t = load_w_rows(wop, wo, D, "wot", DT)
        with tc.tile_pool(name="ps3", bufs=2, space="PSUM") as ps3, \
             tc.tile_pool(name="sqA", bufs=2) as sqA, \
             tc.tile_pool(name="tmA", bufs=4) as tmA:
            a_str = a2a_recv(t1out, atmpp, astrp, "a1")
            out_project_strip(wo_t, bo_t, a_str, z32, ps3, "o1")
            layer_norm_strip(z32, z16, 0, ps3, sqA, tmA, "l1")
        # cross-Q: project full q on my strip, then AllToAll to head-shard it
        with tc.tile_pool(name="wqxp", bufs=DT) as wqxp, \
             tc.tile_pool(name="qfp", bufs=DT) as qfp, \
             tc.tile_pool(name="ps5q", bufs=2, space="PSUM") as ps5q:
            wqxf_t = load_w_rows(wqxp, wqx, D, "wqxf", DT)
            qf = []
            for pt in range(DT):
                ps = ps5q.tile([128, CW], F32, name="qfps", tag="qf", bufs=2)
                for d in range(DT):
                    nc.tensor.matmul(
                        out=ps[:], lhsT=wqxf_t[d][:, pt * 128:(pt + 1) * 128],
                        rhs=z16[d][:],
                        start=(d == 0), stop=(d == DT - 1))
                t = qfp.tile([128, SW], BF16, name="qf", tag="qf", bufs=DT)
                nc.scalar.activation(out=t[:], in_=ps[:], func=AF.Identity,
                                     bias=bqx_t[pt][:], scale=1.0)
                qf.append(t)
            for j in range(2 * G):
                for u in range(2):
                    DMA_ENG[(2 * j + u) % len(DMA_ENG)].dma_start(
                        out=q2in[j * 256 + u * 128: j * 256 + (u + 1) * 128, :],
                        in_=qf[2 * (j % G) + u][:])
            nc.gpsimd.collective_compute(
                "AllToAll", ALU.bypass, replica_groups=GROUP8,
                ins=[q2in.opt()], outs=[q2out.opt()])
            # remaining cross-V work fills the AllToAll window
            with tc.tile_pool(name="psxB", bufs=2, space="PSUM") as psxB, \
                 tc.tile_pool(name="psxvB", bufs=2, space="PSUM") as psxvB:
                ckv_steps(psxB, psxvB, 6, 12)

        # ---- P6: assemble head-sharded cross Q from the A2A ----
        q_x = [qkp.tile([128, S], BF16, name="qx", tag="qk", bufs=8)
               for _ in range(2)]
        for u in range(2):
            for c in range(G):
                top = atmpp.tile([128, SW], BF16, name="qxt", tag="atmp",
                                 bufs=8)
                bot = atmpp.tile([128, SW], BF16, name="qxb", tag="atmp",
                                 bufs=8)
                DMA_ENG[c % len(DMA_ENG)].dma_start(
                    out=top[:],
                    in_=q2out[256 * c + 128 * u: 256 * c + 128 * (u + 1), :])
                DMA_ENG[(c + 1) % len(DMA_ENG)].dma_start(
                    out=bot[:],
                    in_=q2out[D + 256 * c + 128 * u:
                              D + 256 * c + 128 * (u + 1), :])
                slc = slice(c * SW, (c + 1) * SW)
                nc.vector.tensor_scalar_mul(out=q_x[u][:, slc], in0=top[:],
                                            scalar1=gmask_t[:, 0:1])
                nc.vector.scalar_tensor_tensor(
                    out=q_x[u][:, slc], in0=bot[:], scalar=gmask_t[:, 1:2],
                    in1=q_x[u][:, slc], op0=ALU.mult, op1=ALU.add)
        with tc.tile_pool(name="pp2", bufs=7) as pp2, \
             tc.tile_pool(name="dn2", bufs=6) as dn2, \
             tc.tile_pool(name="ps6s", bufs=4, space="PSUM") as ps6s, \
             tc.tile_pool(name="ps6v", bufs=4, space="PSUM") as ps6v:
            a_cx = attention(apool, pp2, dn2, q_x, kx, vx, M, False,
                             ps6s, ps6v, "cx")

        attn_stack.close()  # frees mem, q/k/v, weights for qkv

        # ---- P7/P8: A2A cross, out-proj, LN2 ----
        a2a_send(a_cx, t3in, t3out)
        wox_t = load_w_rows(wop, wox, D, "wot", DT)
        with tc.tile_pool(name="ps7", bufs=2, space="PSUM") as ps7, \
             tc.tile_pool(name="sqB", bufs=2) as sqB, \
             tc.tile_pool(name="tmB", bufs=4) as tmB:
            ax_str = a2a_recv(t3out, atmpp, astrp, "a3")
            out_project_strip(wox_t, box_t, ax_str, z32, ps7, "o2")
            layer_norm_strip(z32, z16, 1, ps7, sqB, tmB, "l2")

        # ---- P9: FFN + LN3 + output ----
        with tc.tile_pool(name="hp", bufs=FT) as hp, \
             tc.tile_pool(name="w1sp", bufs=6) as w1sp, \
             tc.tile_pool(name="w2p", bufs=6) as w2p, \
             tc.tile_pool(name="sqC", bufs=2) as sqC, \
             tc.tile_pool(name="tmC", bufs=4) as tmC:
            h_t = [None] * FT
            # FF1 in f-blocks of 8: stream w1 row-tiles [128(d), 1024(f-blk)]
            with tc.tile_pool(name="ps9a", bufs=8, space="PSUM") as ps9a:
                for fb in range(FT // 8):
                    f1ps = [ps9a.tile([128, CW], F32, name=f"f1ps{i}",
                                      tag=f"f1ps{i}", bufs=1)
                            for i in range(8)]
                    for d in range(DT):
                        w1t = w1sp.tile([128, 1024], BF16, name="w1t", tag="w1",
                                        bufs=6)
                        DMA_ENG[d % len(DMA_ENG)].dma_start(
                            out=w1t[:],
                            in_=w1[d * 128:(d + 1) * 128,
                                   fb * 1024:(fb + 1) * 1024])
                        for i in range(8):
                            nc.tensor.matmul(
                                out=f1ps[i][:],
                                lhsT=w1t[:, i * 128:(i + 1) * 128],
                                rhs=z16[d][:],
                                start=(d == 0), stop=(d == DT - 1))
                    for i in range(8):
                        f = fb * 8 + i
                        ht = hp.tile([128, CW], BF16, name="ht", tag="h",
                                     bufs=FT)
                        nc.scalar.activation(out=ht[:], in_=f1ps[i][:],
                                             func=AF.Relu, bias=b1_t[f][:],
                                             scale=1.0)
                        h_t[f] = ht
            # FF2: f-outer accumulation into 8 concurrent psum banks
            with tc.tile_pool(name="ps9b", bufs=8, space="PSUM") as ps9b:
                f2ps = [ps9b.tile([128, CW], F32, name=f"f2ps{d}",
                                  tag=f"f2ps{d}", bufs=1) for d in range(DT)]
                for f in range(FT):
                    w2t = w2p.tile([128, D], BF16, name="w2t", tag="w2", bufs=6)
                    DMA_ENG[f % len(DMA_ENG)].dma_start(
                        out=w2t[:], in_=w2[f * 128:(f + 1) * 128, :])
                    for d in range(DT):
                        nc.tensor.matmul(
                            out=f2ps[d][:], lhsT=w2t[:, d * 128:(d + 1) * 128],
                            rhs=h_t[f][:],
                            start=(f == 0), stop=(f == FT - 1))
                for d in range(DT):
                    nc.vector.scalar_tensor_tensor(
                        out=z32[d][:], in0=f2ps[d][:], scalar=b2_t[d][:],
                        in1=z32[d][:], op0=ALU.add, op1=ALU.add)
            with tc.tile_pool(name="ps9c", bufs=2, space="PSUM") as ps9c:
                layer_norm_strip(z32, None, 2, ps9c, sqC, tmC, "l3")
            for d in range(DT):
                DMA_ENG[d % len(DMA_ENG)].dma_start(
                    out=outT[d * 128:(d + 1) * 128, :], in_=z32[d][:])

    nc.finalize()
    return nc


def _get_nc(S, M, causal):
    key = (S, M, causal)
    if key not in _nc_cache:
        _nc_cache[key] = _build(S, M, causal)
    return _nc_cache[key]


def _prep_inputs(c, S, M, tgt, memory, Wqkv, bqkv, Wo_sa, bo_sa, Wq, bq, Wk, bk,
                 Wv, bv, Wo_cx, bo_cx, W1, b1, W2, b2, g_mha, bn_mha, g_crx,
                 bn_crx, g_ffn, bn_ffn):
    r, b = c % G, c // G
    hsl = slice(r * DC, (r + 1) * DC)
    f32 = np.float32
    bf16 = ml_dtypes.bfloat16

    def aug_v(wv_c, bv_c):
        wva = np.zeros((D, DCA), f32)
        bva = np.zeros((1, DCA), f32)
        for h in range(HL):
            wva[:, h * 65:h * 65 + 64] = wv_c[:, h * 64:(h + 1) * 64]
            bva[0, h * 65:h * 65 + 64] = bv_c[h * 64:(h + 1) * 64]
            bva[0, h * 65 + 64] = 1.0
        return wva, bva

    scale = np.float32(1.0 / np.sqrt(HD))
    wqkv_h = Wqkv.reshape(D, H, 3 * HD)
    bqkv_h = bqkv.reshape(H, 3 * HD)
    gh = slice(r * HL, (r + 1) * HL)
    wq_sa = wqkv_h[:, gh, 0:HD].reshape(D, DC) * scale
    wk_sa = wqkv_h[:, gh, HD:2 * HD].reshape(D, DC)
    wv_sa = wqkv_h[:, gh, 2 * HD:3 * HD].reshape(D, DC)
    bq_sa = bqkv_h[gh, 0:HD].reshape(DC) * scale
    bk_sa = bqkv_h[gh, HD:2 * HD].reshape(DC)
    bv_sa = bqkv_h[gh, 2 * HD:3 * HD].reshape(DC)
    wva_sa, bva_sa = aug_v(wv_sa, bv_sa)
    wvx_c, bvx_c = aug_v(Wv[:, hsl], bv[hsl])
    xT_full = np.ascontiguousarray(tgt[b].T)
    g0 = np.float32(1.0 if b == 0 else 0.0)
    gmask = np.broadcast_to(np.array([g0, 1.0 - g0], f32), (128, 2))
    m4 = np.zeros(4, f32)
    m4[(0 if r < 2 else 2) + b] = 1.0
    gmask4 = np.broadcast_to(m4, (128, 4))
    return {
        "xT": xT_full.astype(bf16),
        "xs": np.ascontiguousarray(xT_full[:, r * SW:(r + 1) * SW], f32),
        "memT": np.ascontiguousarray(memory[b].T).astype(bf16),
        "wq": wq_sa.astype(bf16),
        "wk": np.ascontiguousarray(wk_sa).astype(bf16),
        "wv": wva_sa.astype(bf16),
        "bq": np.ascontiguousarray(bq_sa.reshape(DC, 1), f32),
        "bk": np.ascontiguousarray(bk_sa.reshape(DC, 1), f32),
        "bva": bva_sa,
        "wo": Wo_sa.astype(bf16),
        "bo": np.ascontiguousarray(bo_sa.reshape(D, 1), f32),
        "wqx": (Wq * scale).astype(bf16),
        "wkx": np.ascontiguousarray(Wk[:, hsl]).astype(bf16),
        "wvx": wvx_c.astype(bf16),
        "bqx": np.ascontiguousarray((bq * scale).reshape(D, 1), f32),
        "bkx": np.ascontiguousarray(bk[hsl].reshape(DC, 1), f32),
        "bvxa": bvx_c,
        "wox": Wo_cx.astype(bf16),
        "box": np.ascontiguousarray(bo_cx.reshape(D, 1), f32),
        "w1": W1.astype(bf16),
        "b1": np.ascontiguousarray(b1.reshape(FF, 1), f32),
        "w2": W2.astype(bf16),
        "b2": np.ascontiguousarray(b2.reshape(D, 1), f32),
        "lng": np.ascontiguousarray(
            np.concatenate([g_mha, g_crx, g_ffn]).reshape(3 * D, 1), f32),
        "lnb": np.ascontiguousarray(
            np.concatenate([bn_mha, bn_crx, bn_ffn]).reshape(3 * D, 1), f32),
        "ones": np.full((128, 128), 1.0 / D, f32),
        "gmask": np.ascontiguousarray(gmask, f32),
        "gmask4": np.ascontiguousarray(gmask4, f32),
    }


def kernel(**inputs):
    tgt = np.asarray(inputs["tgt"], np.float32)
    memory = np.asarray(inputs["memory"], np.float32)
    mask = np.asarray(inputs["tgt_mask"])
    S, M = tgt.shape[1], memory.shape[1]

    if mask.any():
        expect = np.triu(np.ones((S, S), bool), 1)
        if not np.array_equal(mask, expect):
            raise NotImplementedError("only causal or empty tgt_mask supported")
        causal = True
    else:
        causal = False

    nc = _get_nc(S, M, causal)
    args = {k: np.asarray(v, np.float32) for k, v in inputs.items()
            if k not in ("tgt", "memory", "tgt_mask")}
    in_maps = [_prep_inputs(c, S, M, tgt, memory, **args) for c in range(NCORES)]

    trace = bool(int(os.environ.get("BASS_KERNEL_TRACE", "0")))
    res = run_bass_kernel_spmd(nc, in_maps, list(range(NCORES)), trace=trace)
    if trace:
        kernel.last_exec_time_ns = res.exec_time_ns
        kernel.last_result = res
    out = np.empty((B, S, D), np.float32)
    for c in range(NCORES):
        r, b = c % G, c // G
        out[b, r * SW:(r + 1) * SW, :] = res.results[c]["outT"].T
    return out



# revision 34
# speedup vs baseline: 1.3350x; 1.3350x over previous
"""Trainium2 Bass kernel for nn_DecoderBlock (self-attn + cross-attn + FFN, post-LN).

Sharding (8 cores = 2 batch groups x 4 cores):
 - Attention is head-parallel within each group (4 heads/core, full S), which
   keeps the causal-mask instruction stream rank-uniform (SPMD).
 - Everything else (out-projections, LayerNorms, FFN) is sequence-parallel:
   each core owns a 512-token strip and uses the FULL Wo/W1/W2 weights, so
   there are no partial sums and no AllReduces.
 - Resharding transitions are 8-core AllToAlls, each split into two 1MB
   halves keyed by head-pair so the first half flies while the second
   head-pair is still being computed.  Receivers combine the two batch-group
   blocks with a per-core 0/1 mask (SPMD: rank variation lives in host data).
 - All matmul operands are bf16 (fp32 PSUM accumulation); the residual/LN
   stream is kept fp32.
 - Engine budget: scalar = exp/rsqrt only; vector = softmax denominators,
   masks, biases, LN arithmetic; gpsimd = collective triggers, A2A recv
   combines, LN squares/copies; sync = broadcast/spill DMAs.
"""

import os
import sys

sys.path.insert(0, "/opt/trn_rl_repo")

from contextlib import ExitStack

import ml_dtypes
import numpy as np

import concourse.bacc as bacc
import concourse.tile as tile
from concourse import mybir
from concourse.bass_utils import run_bass_kernel_spmd

F32R = mybir.dt.float32r
F32 = mybir.dt.float32
BF16 = mybir.dt.bfloat16
AF = mybir.ActivationFunctionType
ALU = mybir.AluOpType

B = 2
D = 1024
H = 16
HD = 64
FF = 4 * D
NCORES = 8
G = 4                  # cores per batch group
HL = H // G            # 4 local heads
DC = HL * HD           # 256 local q/k/v features
DCA = HL * (HD + 1)    # 260: V augmented with a ones column per head
SW = 512               # tokens per core strip
CW = 512               # free-dim chunk width
DT = D // 128          # 8 feature partition-tiles
FT = FF // 128         # 32 ffn partition-tiles
GROUP8 = [[0, 1, 2, 3, 4, 5, 6, 7]]

# bf16 weight pack column offsets (per-d blocks of cols each)
WO_Q = 0
WO_K = WO_Q + DC
WO_V = WO_K + DC
WO_KX = WO_V + DCA
WO_VX = WO_KX + DC
WCOLS = WO_VX + DCA    # 1292 cols per d

# f32 constant pack column offsets
CO_ONES = 0            # [128,128] ones/D
CO_GM = CO_ONES + 128  # gmask 2
CO_GMQ = CO_GM + 2     # (unused spare) 4
CO_BQ = CO_GMQ + 4     # bq 2
CO_BK = CO_BQ + 2      # bk 2
CO_BQX = CO_BK + 2     # bqx 8
CO_BKX = CO_BQX + DT   # bkx 2
CO_BO = CO_BKX + 2     # bo 8
CO_BOX = CO_BO + DT    # box 8
CO_B1 = CO_BOX + DT    # b1 32
CO_B2 = CO_B1 + FT     # b2 8
CO_LNG = CO_B2 + DT    # lng 24
CO_LNB = CO_LNG + 3 * DT  # lnb 24
CO_BVA = CO_LNB + 3 * DT  # bva 260 (partition-replicated)
CO_BVXA = CO_BVA + DCA    # bvxa 260
CCOLS = CO_BVXA + DCA

_nc_cache = {}


def _build(S, M, causal):
    nc = bacc.Bacc(None, target_bir_lowering=False, num_devices=NCORES)
    NCH = S // CW          # 4 query chunks (full S)
    ST = S // 128          # 16 key tiles (self)

    dp = nc.declare_dram_parameter
    xpk = dp("xpk", [128, DT * S], BF16, isOutput=False)
    mpk = dp("mpk", [128, DT * M], BF16, isOutput=False)
    xspk = dp("xspk", [128, DT * SW], F32R, isOutput=False)
    wpk = dp("wpk", [128, DT * WCOLS], BF16, isOutput=False)
    cpk = dp("cpk", [128, CCOLS], F32, isOutput=False)
    ones = dp("ones", [128, 128], F32R, isOutput=False)
    mskpk = dp("mskpk", [128, 4 * CW], BF16, isOutput=False)
    wo = dp("wo", [D, D], BF16, isOutput=False)
    wqx = dp("wqx", [D, D], BF16, isOutput=False)
    wox = dp("wox", [D, D], BF16, isOutput=False)
    w1 = dp("w1", [D, FF], BF16, isOutput=False)
    w2 = dp("w2", [FF, D], BF16, isOutput=False)
    outT = dp("outT", [D, SW], F32R, isOutput=True)
    DEBUG = bool(int(os.environ.get("BASS_DEBUG_DUMP", "0")))
    if DEBUG:
        dbg_asa = dp("dbg_asa", [2 * 128, S], BF16, isOutput=True)
        dbg_astr = dp("dbg_astr", [D, SW], BF16, isOutput=True)
        dbg_z1 = dp("dbg_z1", [D, SW], F32R, isOutput=True)
        dbg_qx = dp("dbg_qx", [2 * 128, S], BF16, isOutput=True)
        dbg_kx = dp("dbg_kx", [2 * 128, M], BF16, isOutput=True)
        dbg_acx = dp("dbg_acx", [2 * 128, S], BF16, isOutput=True)
        dbg_ax = dp("dbg_ax", [D, SW], BF16, isOutput=True)
        dbg_z2 = dp("dbg_z2", [D, SW], F32R, isOutput=True)

    with tile.TileContext(nc) as tc, ExitStack() as st:
        ep = st.enter_context
        constp = ep(tc.tile_pool(name="const", bufs=1))
        dramp = ep(tc.tile_pool(name="dram", bufs=1, space="DRAM"))

        # gpsimd's queue blocks behind the collectives-prelude barrier and
        # behind any in-flight collective trigger, so anything that must
        # move while an AllToAll is airborne goes on the HWDGE queues.
        DMA_ENG = [nc.sync, nc.gpsimd, nc.scalar]   # collective-free phases
        LOADQ = [nc.sync, nc.scalar]                # always safe

        # ---- DRAM bounce buffers for collectives (1MB halves) ----
        t1in = [dramp.tile([D, SW], BF16, name=f"t1in{h}") for h in range(2)]
        t1out = [dramp.tile([D, SW], BF16, name=f"t1out{h}") for h in range(2)]
        q2in = [dramp.tile([D, SW], BF16, name=f"q2in{h}") for h in range(2)]
        q2out = [dramp.tile([D, SW], BF16, name=f"q2out{h}") for h in range(2)]
        t3in = [dramp.tile([D, SW], BF16, name=f"t3in{h}") for h in range(2)]
        t3out = [dramp.tile([D, SW], BF16, name=f"t3out{h}") for h in range(2)]

        # ---- packed constants: one DMA each ----
        cpk_t = constp.tile([128, CCOLS], F32, name="cpk_t")
        nc.scalar.dma_start(out=cpk_t[:], in_=cpk[:, :])
        msk_t = constp.tile([128, 4 * CW], BF16, name="msk_t")
        nc.scalar.dma_start(out=msk_t[:], in_=mskpk[:, :])
        eps_t = constp.tile([128, 1], F32, name="eps_t")
        nc.vector.memset(eps_t[:], 1e-5)
        ones_t = constp.tile([128, 128], F32R, name="ones_t")
        nc.scalar.dma_start(out=ones_t[:], in_=ones[:, :])
        gmask_t = cpk_t[:, CO_GM:CO_GM + 2]
        col = lambda off, i: cpk_t[:, off + i:off + i + 1]
        bq_t = [col(CO_BQ, i) for i in range(2)]
        bk_t = [col(CO_BK, i) for i in range(2)]
        bqx_t = [col(CO_BQX, i) for i in range(DT)]
        bkx_t = [col(CO_BKX, i) for i in range(2)]
        bo_t = [col(CO_BO, i) for i in range(DT)]
        box_t = [col(CO_BOX, i) for i in range(DT)]
        b1_t = [col(CO_B1, i) for i in range(FT)]
        b2_t = [col(CO_B2, i) for i in range(DT)]
        lng_t = [col(CO_LNG, i) for i in range(3 * DT)]
        lnb_t = [col(CO_LNB, i) for i in range(3 * DT)]
        bva_t = cpk_t[:, CO_BVA:CO_BVA + DCA]
        bvxa_t = cpk_t[:, CO_BVXA:CO_BVXA + DCA]

        # ================= helpers =================
        def project_qk(qt_pair, w_sl, b_tiles, src_tiles, psum, tag):
            """q/k projection: 2x [128, S] packed tiles (2 heads each)."""
            for sc in range(NCH):
                sl = slice(sc * CW, (sc + 1) * CW)
                for t in range(2):
                    ps = psum.tile([128, CW], F32, name=f"{tag}ps", tag="qkps",
                                   bufs=2)
                    for d in range(DT):
                        nc.tensor.matmul(
                            out=ps[:],
                            lhsT=w_sl[d][:, t * 128:(t + 1) * 128],
                            rhs=src_tiles[d][:, sl],
                            start=(d == 0), stop=(d == DT - 1),
                        )
                    nc.vector.tensor_scalar_add(
                        out=qt_pair[t][:, sl], in0=ps[:],
                        scalar1=b_tiles[t][:])

        def project_v(vpool, w_sl, bias_bc, src_tiles, psum, n_tok, tag):
            """v projection: n_tok/128 tiles of [128 tok, DCA]."""
            vs = []
            for s_t in range(n_tok // 128):
                ps = psum.tile([128, DCA], F32, name=f"{tag}ps", tag="vps",
                               bufs=2)
                for d in range(DT):
                    nc.tensor.matmul(
                        out=ps[:],
                        lhsT=src_tiles[d][:, s_t * 128:(s_t + 1) * 128],
                        rhs=w_sl[d][:],
                        start=(d == 0), stop=(d == DT - 1))
                vt = vpool.tile([128, DCA], BF16, name=f"{tag}v", tag="vs",
                                bufs=2 * ST)
                nc.vector.tensor_tensor(out=vt[:], in0=ps[:], in1=bias_bc[:],
                                        op=ALU.add)
                vs.append(vt)
            return vs

        def attention(apool, ppool, dpool, q_tiles, k_tiles, v_tiles, n_keys,
                      use_mask, psc, ppv, tag, before_ti=None, after_ti=None):
            """Head-sharded attention, ti-OUTER loop; returns 2 packed
            [128, S] bf16 tiles (2 heads each).  before_ti(ti)/after_ti(ti)
            are hooks for assembling q just-in-time and spilling/triggering
            the output AllToAll half per head-pair."""
            a_packed = [apool.tile([128, S], BF16, name=f"{tag}{t}", tag="attn",
                                   bufs=2) for t in range(2)]
            kt_total = n_keys // 128
            kpc = CW // 128
            for ti in range(HL // 2):
                if before_ti is not None:
                    before_ti(ti)
                for qc in range(NCH):
                    sl = slice(qc * CW, (qc + 1) * CW)
                    kts = range(min(kt_total, kpc * (qc + 1)) if use_mask
                                else kt_total)
                    n_kt = len(kts)
                    pv_ps = [ppv.tile([65, CW], F32, name=f"{tag}pv{par}",
                                      tag="pvps", bufs=4) for par in range(2)]

                    def emit_scores(kt):
                        s_ps = psc.tile([128, 2 * CW], F32, name=f"{tag}s",
                                        tag="scps", bufs=2)
                        for par in range(2):
                            nc.tensor.matmul(
                                out=s_ps[:, par * CW:(par + 1) * CW],
                                lhsT=k_tiles[ti][par * 64:(par + 1) * 64,
                                                 kt * 128:(kt + 1) * 128],
                                rhs=q_tiles[ti][par * 64:(par + 1) * 64, sl],
                                start=True, stop=True,
                                tile_position=(64 * par, 0),
                            )
                        return s_ps

                    def emit_pv(s_ps, kt, i):
                        p_t = ppool.tile([128, 2 * CW], BF16, name=f"{tag}p",
                                         tag="p", bufs=4)
                        nc.scalar.activation(out=p_t[:], in_=s_ps[:],
                                             func=AF.Exp)
                        masked = use_mask and kt >= kpc * qc
                        if masked:
                            mi = kt - kpc * qc
                            p_m = ppool.tile([128, 2 * CW], BF16,
                                             name=f"{tag}pm", tag="p", bufs=4)
                            for par in range(2):
                                nc.vector.tensor_tensor(
                                    out=p_m[:, par * CW:(par + 1) * CW],
                                    in0=p_t[:, par * CW:(par + 1) * CW],
                                    in1=msk_t[:, mi * CW:(mi + 1) * CW],
                                    op=ALU.mult)
                            p_use = p_m
                        else:
                            p_use = p_t
                        for par in range(2):
                            h = 2 * ti + par
                            nc.tensor.matmul(
                                out=pv_ps[par][:],
                                lhsT=v_tiles[kt][:, h * 65:(h + 1) * 65],
                                rhs=p_use[:, par * CW:(par + 1) * CW],
                                start=(i == 0), stop=(i == n_kt - 1),
                            )

                    # one-step lookahead: scores of kt+1 are emitted before
                    # the exp-gated PV of kt so the PE never queues behind
                    # a semaphore wait on the scalar engine
                    prev = None
                    for i, kt in enumerate(kts):
                        s_ps = emit_scores(kt)
                        if prev is not None:
                            emit_pv(*prev)
                        prev = (s_ps, kt, i)
                    emit_pv(*prev)
                    for par in range(2):
                        # softmax denominator: copy row 64 out of PSUM,
                        # partition-broadcast, approx-reciprocal, multiply
                        # (reciprocal_approx_fast misbehaves on [1, N] APs
                        # at a partition offset, so it runs post-broadcast)
                        drc = dpool.tile([65, CW], F32, name=f"{tag}drc",
                                         tag="drc", bufs=2)
                        nc.vector.tensor_copy(out=drc[64:65, :],
                                              in_=pv_ps[par][64:65, :])
                        dn0 = dpool.tile([1, CW], F32, name=f"{tag}dn0",
                                         tag="dn0", bufs=2)
                        nc.sync.dma_start(out=dn0[0:1, :], in_=drc[64:65, :])
                        db = dpool.tile([64, CW], F32, name=f"{tag}db",
                                        tag="db", bufs=2)
                        nc.gpsimd.partition_broadcast(db[:], dn0[0:1, :])
                        dbr = dpool.tile([64, CW], F32, name=f"{tag}dbr",
                                         tag="dbr", bufs=2)
                        nc.vector.reciprocal_approx_fast(out=dbr[:], in_=db[:])
                        nc.vector.tensor_tensor(
                            out=a_packed[ti][par * 64:(par + 1) * 64, sl],
                            in0=pv_ps[par][0:64, :], in1=dbr[:], op=ALU.mult)
                if after_ti is not None:
                    after_ti(ti, a_packed[ti])
            return a_packed

        def a2a_spill_strip(a_tile, zin, j):
            """Spill dest-rank j's strip of one head-pair tile to DRAM."""
            LOADQ[j % len(LOADQ)].dma_start(
                out=zin[j * 128:(j + 1) * 128, :],
                in_=a_tile[:, (j % G) * SW:(j % G + 1) * SW])

        def a2a_trigger(zin, zout):
            nc.gpsimd.collective_compute(
                "AllToAll", ALU.bypass, replica_groups=GROUP8,
                ins=[zin.opt()], outs=[zout.opt()])

        def a2a_send(a_tile, zin, zout):
            for j in range(2 * G):
                a2a_spill_strip(a_tile, zin, j)
            a2a_trigger(zin, zout)

        def a2a_recv2(zouts, tmp_pool, a_pool, tag):
            """Strip assembly: ft feature-tile = head pair (ft//2 group-rank,
            ft%2 head-pair); masked 2-way combine of the batch-group blocks."""
            a_str = []
            for ft in range(DT):
                g, u = ft // 2, ft % 2
                z = zouts[u]
                top = tmp_pool.tile([128, SW], BF16, name=f"{tag}t", tag="atmp",
                                    bufs=8)
                bot = tmp_pool.tile([128, SW], BF16, name=f"{tag}b", tag="atmp",
                                    bufs=8)
                LOADQ[ft % len(LOADQ)].dma_start(
                    out=top[:], in_=z[g * 128:(g + 1) * 128, :])
                LOADQ[(ft + 1) % len(LOADQ)].dma_start(
                    out=bot[:], in_=z[(G + g) * 128:(G + g + 1) * 128, :])
                a = a_pool.tile([128, SW], BF16, name=f"{tag}a", tag="astr",
                                bufs=DT)
                nc.vector.tensor_scalar_mul(out=a[:], in0=top[:],
                                            scalar1=gmask_t[:, 0:1])
                nc.vector.scalar_tensor_tensor(
                    out=a[:], in0=bot[:], scalar=gmask_t[:, 1:2], in1=a[:],
                    op0=ALU.mult, op1=ALU.add)
                a_str.append(a)
            return a_str

        def out_project_strip(wo_tiles, b_tiles, a_str, z32, psum, tag):
            """z32[d] = wo.T @ a_str + bias + z32  (residual add in place)."""
            for d in range(DT):
                ps = psum.tile([128, CW], F32, name=f"{tag}ps", tag="ops",
                               bufs=2)
                for kt in range(DT):
                    nc.tensor.matmul(
                        out=ps[:], lhsT=wo_tiles[kt][:, d * 128:(d + 1) * 128],
                        rhs=a_str[kt][:],
                        start=(kt == 0), stop=(kt == DT - 1))
                nc.vector.scalar_tensor_tensor(
                    out=z32[d][:], in0=ps[:], scalar=b_tiles[d][:],
                    in1=z32[d][:], op0=ALU.add, op1=ALU.add)

        def layer_norm_strip(z32, z16, ln_idx, psum, sqpool, tmppool, tag):
            """Post-LN on the [D, SW] fp32 strip; writes bf16 copy z16."""
            mps = psum.tile([128, CW], F32, name=f"{tag}m", tag="lnps", bufs=2)
            for d in range(DT):
                nc.tensor.matmul(out=mps[:], lhsT=ones_t[:],
                                 rhs=z32[d][:],
                                 start=(d == 0), stop=(d == DT - 1))
            mu = tmppool.tile([128, CW], F32, name=f"{tag}mu", tag="mu", bufs=1)
            nc.vector.tensor_copy(out=mu[:], in_=mps[:])
            qps = psum.tile([128, CW], F32, name=f"{tag}q", tag="lnps", bufs=2)
            for d in range(DT):
                sq = sqpool.tile([128, CW], F32R, name=f"{tag}sq", tag="sq",
                                 bufs=2)
                nc.gpsimd.tensor_tensor(out=sq[:], in0=z32[d][:],
                                        in1=z32[d][:], op=ALU.mult)
                nc.tensor.matmul(out=qps[:], lhsT=ones_t[:], rhs=sq[:],
                                 start=(d == 0), stop=(d == DT - 1))
            var = tmppool.tile([128, CW], F32, name=f"{tag}v", tag="tv", bufs=1)
            nc.vector.tensor_tensor(out=var[:], in0=mu[:], in1=mu[:],
                                    op=ALU.mult)
            nc.vector.tensor_tensor(out=var[:], in0=qps[:], in1=var[:],
                                    op=ALU.subtract)
            std = tmppool.tile([128, CW], F32, name=f"{tag}sd", tag="std",
                               bufs=1)
            nc.scalar.activation(out=std[:], in_=var[:], func=AF.Sqrt,
                                 bias=eps_t[:], scale=1.0)
            rstd = tmppool.tile([128, CW], F32, name=f"{tag}r", tag="rstd",
                                bufs=1)
            nc.vector.reciprocal_approx_fast(out=rstd[:], in_=std[:])
            for d in range(DT):
                xm = tmppool.tile([128, CW], F32, name=f"{tag}x", tag="xm",
                                  bufs=2)
                nc.gpsimd.tensor_tensor(out=xm[:], in0=z32[d][:], in1=mu[:],
                                        op=ALU.subtract)
                nc.vector.tensor_tensor(out=xm[:], in0=xm[:], in1=rstd[:],
                                        op=ALU.mult)
                nc.vector.tensor_scalar(
                    out=z32[d][:], in0=xm[:],
                    scalar1=lng_t[ln_idx * DT + d][:],
                    scalar2=lnb_t[ln_idx * DT + d][:],
                    op0=ALU.mult, op1=ALU.add)
                if z16 is not None:
                    nc.scalar.copy(out=z16[d][:], in_=z32[d][:])

        # ================= pipeline =================
        zp = ep(tc.tile_pool(name="zp", bufs=1))
        z16p = ep(tc.tile_pool(name="z16p", bufs=DT))
        wop = ep(tc.tile_pool(name="wop", bufs=DT))
        apool = ep(tc.tile_pool(name="apool", bufs=2))
        atmpp = ep(tc.tile_pool(name="atmpp", bufs=8))
        astrp = ep(tc.tile_pool(name="astrp", bufs=DT))

        attn_stack = ExitStack()
        ap2 = attn_stack.enter_context
        mpool = ap2(tc.tile_pool(name="memp", bufs=1))
        qkp = ap2(tc.tile_pool(name="qk", bufs=8))
        vp = ap2(tc.tile_pool(name="vp", bufs=2 * ST))
        wqkp = ap2(tc.tile_pool(name="wqk", bufs=1))

        # ---- P0/P1: loads + self QKV ----
        with tc.tile_pool(name="xp", bufs=1) as xpool:
            x_t = xpool.tile([128, DT * S], BF16, name="xt", tag="x", bufs=1)
            half = DT * S // 2
            nc.sync.dma_start(out=x_t[:, :half], in_=xpk[:, :half])
            nc.sync.dma_start(out=x_t[:, half:], in_=xpk[:, half:])
            x_fm = [x_t[:, d * S:(d + 1) * S] for d in range(DT)]

            w_t = wqkp.tile([128, DT * WCOLS], BF16, name="wt", tag="w",
                            bufs=1)
            nc.scalar.dma_start(out=w_t[:], in_=wpk[:, :])
            wsl = lambda off, cols: [
                w_t[:, d * WCOLS + off:d * WCOLS + off + cols]
                for d in range(DT)]
            wq_t = wsl(WO_Q, DC)
            wk_t = wsl(WO_K, DC)
            wv_t = wsl(WO_V, DCA)
            wkx_t = wsl(WO_KX, DC)
            wvx_t = wsl(WO_VX, DCA)

            m_t = mpool.tile([128, DT * M], BF16, name="mt", tag="m", bufs=1)
            mhalf = DT * M // 2
            nc.scalar.dma_start(out=m_t[:, :mhalf], in_=mpk[:, :mhalf])
            nc.sync.dma_start(out=m_t[:, mhalf:], in_=mpk[:, mhalf:])
            m_fm = [m_t[:, d * M:(d + 1) * M] for d in range(DT)]

            z_t = zp.tile([128, DT * SW], F32R, name="z32", tag="z32", bufs=1)
            nc.sync.dma_start(out=z_t[:], in_=xspk[:, :])
            z32 = [z_t[:, d * SW:(d + 1) * SW] for d in range(DT)]
            z16 = [z16p.tile([128, SW], BF16, name="z16", tag="z16", bufs=DT)
                   for _ in range(DT)]

            q_s = [qkp.tile([128, S], BF16, name="qs", tag="qk", bufs=8)
                   for _ in range(2)]
            k_s = [qkp.tile([128, S], BF16, name="ks", tag="qk", bufs=8)
                   for _ in range(2)]
            with tc.tile_pool(name="ps1", bufs=2, space="PSUM") as ps1, \
                 tc.tile_pool(name="ps1v", bufs=2, space="PSUM") as ps1v:
                project_qk(q_s, wq_t, bq_t, x_fm, ps1, "qs")
                project_qk(k_s, wk_t, bk_t, x_fm, ps1, "ks")
                v_s = project_v(vp, wv_t, bva_t, x_fm, ps1v, S, "vs")

        # x freed here; z32 holds the residual strip.

        # ---- P2/P3: self attention with mid-flight A2A halves ----
        kx = [qkp.tile([128, M], BF16, name="kx", tag="qk", bufs=8)
              for _ in range(2)]
        vx = []

        def ckv_kx(psx, lo, hi):
            for sc in range(lo, hi):
                sl = slice(sc * CW, (sc + 1) * CW)
                for t in range(2):
                    ps = psx.tile([128, CW], F32, name="kxps", tag="kxps",
                                  bufs=2)
                    for d in range(DT):
                        nc.tensor.matmul(
                            out=ps[:],
                            lhsT=wkx_t[d][:, t * 128:(t + 1) * 128],
                            rhs=m_fm[d][:, sl],
                            start=(d == 0), stop=(d == DT - 1))
                    nc.vector.tensor_scalar_add(out=kx[t][:, sl], in0=ps[:],
                                                scalar1=bkx_t[t][:])

        def ckv_vx(psxv, lo, hi):
            for s_t in range(lo, hi):
                ps = psxv.tile([128, DCA], F32, name="vxps", tag="vxps",
                               bufs=2)
                for d in range(DT):
                    nc.tensor.matmul(
                        out=ps[:],
                        lhsT=m_fm[d][:, s_t * 128:(s_t + 1) * 128],
                        rhs=wvx_t[d][:],
                        start=(d == 0), stop=(d == DT - 1))
                vt = vp.tile([128, DCA], BF16, name="vx", tag="vs",
                             bufs=2 * ST)
                nc.vector.tensor_tensor(out=vt[:], in0=ps[:],
                                        in1=bvxa_t[:], op=ALU.add)
                vx.append(vt)

        def sa_after_ti(ti, a_tile):
            if DEBUG:
                nc.sync.dma_start(out=dbg_asa[ti * 128:(ti + 1) * 128, :],
                                  in_=a_tile[:])
            for j in range(2 * G):
                a2a_spill_strip(a_tile, t1in[ti], j)
            a2a_trigger(t1in[ti], t1out[ti])

        with tc.tile_pool(name="pp1", bufs=4) as pp1, \
             tc.tile_pool(name="dn1", bufs=6) as dn1, \
             tc.tile_pool(name="ps2s", bufs=2, space="PSUM") as ps2s, \
             tc.tile_pool(name="ps2v", bufs=4, space="PSUM") as ps2v:
            attention(apool, pp1, dn1, q_s, k_s, v_s, S, causal,
                      ps2s, ps2v, "sa", after_ti=sa_after_ti)

        # ---- P4/P5: cross-K fills the A2A#1b window; recv; oproj; LN1 ----
        with tc.tile_pool(name="psx", bufs=2, space="PSUM") as psx:
            ckv_kx(psx, 0, NCH)
        wo_t = []
        for k in range(DT):
            t = wop.tile([128, D], BF16, name="wot", tag="wot", bufs=DT)
            LOADQ[k % len(LOADQ)].dma_start(
                out=t[:], in_=wo[k * 128:(k + 1) * 128, :])
            wo_t.append(t)
        with tc.tile_pool(name="ps3", bufs=2, space="PSUM") as ps3, \
             tc.tile_pool(name="sqA", bufs=2) as sqA, \
             tc.tile_pool(name="tmA", bufs=4) as tmA:
            a_str = a2a_recv2(t1out, atmpp, astrp, "a1")
            if DEBUG:
                for ft in range(DT):
                    nc.sync.dma_start(
                        out=dbg_astr[ft * 128:(ft + 1) * 128, :],
                        in_=a_str[ft][:])
            out_project_strip(wo_t, bo_t, a_str, z32, ps3, "o1")
            layer_norm_strip(z32, z16, 0, ps3, sqA, tmA, "l1")
            if DEBUG:
                for d in range(DT):
                    nc.sync.dma_start(
                        out=dbg_z1[d * 128:(d + 1) * 128, :], in_=z32[d][:])

        # cross-Q: project full q on my strip, then AllToAll halves to
        # head-shard it; remaining cross-V fills the first half's window
        with tc.tile_pool(name="wqxp", bufs=DT) as wqxp, \
             tc.tile_pool(name="qfp", bufs=DT) as qfp, \
             tc.tile_pool(name="ps5q", bufs=2, space="PSUM") as ps5q:
            wqxf_t = []
            for k in range(DT):
                t = wqxp.tile([128, D], BF16, name="wqxf", tag="wqxf", bufs=DT)
                LOADQ[k % len(LOADQ)].dma_start(
                    out=t[:], in_=wqx[k * 128:(k + 1) * 128, :])
                wqxf_t.append(t)
            qf = []
            for pt in range(DT):
                ps = ps5q.tile([128, CW], F32, name="qfps", tag="qf", bufs=2)
                for d in range(DT):
                    nc.tensor.matmul(
                        out=ps[:], lhsT=wqxf_t[d][:, pt * 128:(pt + 1) * 128],
                        rhs=z16[d][:],
                        start=(d == 0), stop=(d == DT - 1))
                t = qfp.tile([128, SW], BF16, name="qf", tag="qf", bufs=DT)
                nc.vector.tensor_scalar_add(out=t[:], in0=ps[:],
                                            scalar1=bqx_t[pt][:])
                qf.append(t)
            for u in range(2):
                for j in range(2 * G):
                    LOADQ[j % len(LOADQ)].dma_start(
                        out=q2in[u][j * 128:(j + 1) * 128, :],
                        in_=qf[2 * (j % G) + u][:])
                a2a_trigger(q2in[u], q2out[u])
                if u == 0:
                    # cross-V fills the first q-A2A window
                    with tc.tile_pool(name="psxv", bufs=2,
                                      space="PSUM") as psxv:
                        ckv_vx(psxv, 0, M // 128)

        # ---- P6: cross attention; q halves assembled just-in-time ----
        q_x = [qkp.tile([128, S], BF16, name="qx", tag="qk", bufs=8)
               for _ in range(2)]

        def cx_before_ti(u):
            for c in range(G):
                top = atmpp.tile([128, SW], BF16, name="qxt", tag="atmp",
                                 bufs=8)
                bot = atmpp.tile([128, SW], BF16, name="qxb", tag="atmp",
                                 bufs=8)
                LOADQ[c % len(LOADQ)].dma_start(
                    out=top[:], in_=q2out[u][c * 128:(c + 1) * 128, :])
                LOADQ[(c + 1) % len(LOADQ)].dma_start(
                    out=bot[:], in_=q2out[u][(G + c) * 128:(G + c + 1) * 128, :])
                slc = slice(c * SW, (c + 1) * SW)
                nc.vector.tensor_scalar_mul(out=q_x[u][:, slc], in0=top[:],
                                            scalar1=gmask_t[:, 0:1])
                nc.vector.scalar_tensor_tensor(
                    out=q_x[u][:, slc], in0=bot[:], scalar=gmask_t[:, 1:2],
                    in1=q_x[u][:, slc], op0=ALU.mult, op1=ALU.add)

        def cx_after_ti(ti, a_tile):
            if DEBUG:
                nc.sync.dma_start(out=dbg_acx[ti * 128:(ti + 1) * 128, :],
                                  in_=a_tile[:])
                nc.sync.dma_start(out=dbg_qx[ti * 128:(ti + 1) * 128, :],
                                  in_=q_x[ti][:])
                nc.sync.dma_start(out=dbg_kx[ti * 128:(ti + 1) * 128, :],
                                  in_=kx[ti][:])
            for j in range(2 * G):
                a2a_spill_strip(a_tile, t3in[ti], j)
            a2a_trigger(t3in[ti], t3out[ti])

        with tc.tile_pool(name="pp2", bufs=4) as pp2, \
             tc.tile_pool(name="dn2", bufs=6) as dn2, \
             tc.tile_pool(name="ps6s", bufs=2, space="PSUM") as ps6s, \
             tc.tile_pool(name="ps6v", bufs=4, space="PSUM") as ps6v:
            attention(apool, pp2, dn2, q_x, kx, vx, M, False,
                      ps6s, ps6v, "cx", before_ti=cx_before_ti,
                      after_ti=cx_after_ti)

        attn_stack.close()  # frees mem, q/k/v, weights for qkv

        # ---- P7/P8: A2A#3b window (warm-keeper matmuls), oproj, LN2 ----
        wox_t = []
        for k in range(DT):
            t = wop.tile([128, D], BF16, name="wot", tag="wot", bufs=DT)
            LOADQ[k % len(LOADQ)].dma_start(
                out=t[:], in_=wox[k * 128:(k + 1) * 128, :])
            wox_t.append(t)
        # keep the PE's HAM clock warm while A2A#3b drains: ~60 throwaway
        # matmuls on resident weight tiles (no data deps, fills the gap)
        with tc.tile_pool(name="wk", bufs=1, space="PSUM") as wkp:
            wps = wkp.tile([128, CW], F32, name="wkps", tag="wkps", bufs=1)
            for i in range(60):
                nc.tensor.matmul(out=wps[:], lhsT=wox_t[i % DT][:, 0:128],
                                 rhs=wox_t[(i + 1) % DT][:, 0:CW],
                                 start=True, stop=True)
        with tc.tile_pool(name="ps7", bufs=2, space="PSUM") as ps7, \
             tc.tile_pool(name="sqB", bufs=2) as sqB, \
             tc.tile_pool(name="tmB", bufs=4) as tmB:
            ax_str = a2a_recv2(t3out, atmpp, astrp, "a3")
            if DEBUG:
                for ft in range(DT):
                    nc.sync.dma_start(
                        out=dbg_ax[ft * 128:(ft + 1) * 128, :],
                        in_=ax_str[ft][:])
            out_project_strip(wox_t, box_t, ax_str, z32, ps7, "o2")
            layer_norm_strip(z32, z16, 1, ps7, sqB, tmB, "l2")
            if DEBUG:
                for d in range(DT):
                    nc.sync.dma_start(
                        out=dbg_z2[d * 128:(d + 1) * 128, :], in_=z32[d][:])

        # ---- P9: FFN + LN3 + output ----
        with tc.tile_pool(name="hp", bufs=FT) as hp, \
             tc.tile_pool(name="w1sp", bufs=6) as w1sp, \
             tc.tile_pool(name="w2p", bufs=6) as w2p, \
             tc.tile_pool(name="sqC", bufs=2) as sqC, \
             tc.tile_pool(name="tmC", bufs=4) as tmC:
            h_t = [None] * FT
            # FF1 in f-blocks of 8: stream w1 row-tiles [128(d), 1024(f-blk)]
            with tc.tile_pool(name="ps9a", bufs=8, space="PSUM") as ps9a:
                for fb in range(FT // 8):
                    f1ps = [ps9a.tile([128, CW], F32, name=f"f1ps{i}",
                                      tag=f"f1ps{i}", bufs=1)
                            for i in range(8)]
                    for d in range(DT):
                        w1t = w1sp.tile([128, 1024], BF16, name="w1t", tag="w1",
                                        bufs=6)
                        DMA_ENG[d % len(DMA_ENG)].dma_start(
                            out=w1t[:],
                            in_=w1[d * 128:(d + 1) * 128,
                                   fb * 1024:(fb + 1) * 1024])
                        for i in range(8):
                            nc.tensor.matmul(
                                out=f1ps[i][:],
                                lhsT=w1t[:, i * 128:(i + 1) * 128],
                                rhs=z16[d][:],
                                start=(d == 0), stop=(d == DT - 1))
                    for i in range(8):
                        f = fb * 8 + i
                        ht = hp.tile([128, CW], BF16, name="ht", tag="h",
                                     bufs=FT)
                        nc.scalar.activation(out=ht[:], in_=f1ps[i][:],
                                             func=AF.Relu, bias=b1_t[f][:],
                                             scale=1.0)
                        h_t[f] = ht
            # FF2: f-outer accumulation into 8 concurrent psum banks
            with tc.tile_pool(name="ps9b", bufs=8, space="PSUM") as ps9b:
                f2ps = [ps9b.tile([128, CW], F32, name=f"f2ps{d}",
                                  tag=f"f2ps{d}", bufs=1) for d in range(DT)]
                for f in range(FT):
                    w2t = w2p.tile([128, D], BF16, name="w2t", tag="w2", bufs=6)
                    DMA_ENG[f % len(DMA_ENG)].dma_start(
                        out=w2t[:], in_=w2[f * 128:(f + 1) * 128, :])
                    for d in range(DT):
                        nc.tensor.matmul(
                            out=f2ps[d][:], lhsT=w2t[:, d * 128:(d + 1) * 128],
                            rhs=h_t[f][:],
                            start=(f == 0), stop=(f == FT - 1))
                for d in range(DT):
                    nc.vector.scalar_tensor_tensor(
                        out=z32[d][:], in0=f2ps[d][:], scalar=b2_t[d][:],
                        in1=z32[d][:], op0=ALU.add, op1=ALU.add)
            with tc.tile_pool(name="ps9c", bufs=2, space="PSUM") as ps9c:
                layer_norm_strip(z32, None, 2, ps9c, sqC, tmC, "l3")
            for d in range(DT):
                DMA_ENG[d % len(DMA_ENG)].dma_start(
                    out=outT[d * 128:(d + 1) * 128, :], in_=z32[d][:])

    nc.finalize()
    return nc


def _get_nc(S, M, causal):
    key = (S, M, causal)
    if key not in _nc_cache:
        _nc_cache[key] = _build(S, M, causal)
    return _nc_cache[key]


def _prep_inputs(c, S, M, tgt, memory, Wqkv, bqkv, Wo_sa, bo_sa, Wq, bq, Wk, bk,
                 Wv, bv, Wo_cx, bo_cx, W1, b1, W2, b2, g_mha, bn_mha, g_crx,
                 bn_crx, g_ffn, bn_ffn):
    r, b = c % G, c // G
    hsl = slice(r * DC, (r + 1) * DC)
    f32 = np.float32
    bf16 = ml_dtypes.bfloat16

    def aug_v(wv_c, bv_c):
        wva = np.zeros((D, DCA), f32)
        bva = np.zeros((1, DCA), f32)
        for h in range(HL):
            wva[:, h * 65:h * 65 + 64] = wv_c[:, h * 64:(h + 1) * 64]
            bva[0, h * 65:h * 65 + 64] = bv_c[h * 64:(h + 1) * 64]
            bva[0, h * 65 + 64] = 1.0
        return wva, bva

    scale = np.float32(1.0 / np.sqrt(HD))
    wqkv_h = Wqkv.reshape(D, H, 3 * HD)
    bqkv_h = bqkv.reshape(H, 3 * HD)
    gh = slice(r * HL, (r + 1) * HL)
    wq_sa = wqkv_h[:, gh, 0:HD].reshape(D, DC) * scale
    wk_sa = wqkv_h[:, gh, HD:2 * HD].reshape(D, DC)
    wv_sa = wqkv_h[:, gh, 2 * HD:3 * HD].reshape(D, DC)
    bq_sa = bqkv_h[gh, 0:HD].reshape(DC) * scale
    bk_sa = bqkv_h[gh, HD:2 * HD].reshape(DC)
    bv_sa = bqkv_h[gh, 2 * HD:3 * HD].reshape(DC)
    wva_sa, bva_sa = aug_v(wv_sa, bv_sa)
    wvx_c, bvx_c = aug_v(Wv[:, hsl], bv[hsl])
    xT_full = np.ascontiguousarray(tgt[b].T)  # [D, S]
    g0 = np.float32(1.0 if b == 0 else 0.0)
    gmask = np.array([g0, 1.0 - g0], f32)

    # packed x: xpk[p, d*S + j] = xT[d*128 + p, j]
    def pack_rows(a, cols):
        # a: [D, cols] -> [128, DT*cols]
        return np.ascontiguousarray(
            a.reshape(DT, 128, cols).transpose(1, 0, 2).reshape(128, DT * cols))

    xpk = pack_rows(xT_full.astype(bf16), S)
    mpk = pack_rows(np.ascontiguousarray(memory[b].T).astype(bf16), M)
    xspk = pack_rows(
        np.ascontiguousarray(xT_full[:, r * SW:(r + 1) * SW], f32), SW)

    # packed weights: wpk[p, d*WCOLS + off + j] = W[d*128 + p, j]
    wblk = np.zeros((D, WCOLS), f32)
    wblk[:, WO_Q:WO_Q + DC] = wq_sa
    wblk[:, WO_K:WO_K + DC] = wk_sa
    wblk[:, WO_V:WO_V + DCA] = wva_sa
    wblk[:, WO_KX:WO_KX + DC] = Wk[:, hsl]
    wblk[:, WO_VX:WO_VX + DCA] = wvx_c
    wpk = pack_rows(wblk.astype(bf16), WCOLS)

    # packed f32 constants
    cpk = np.zeros((128, CCOLS), f32)
    cpk[:, CO_ONES:CO_ONES + 128] = 1.0 / D
    cpk[:, CO_GM:CO_GM + 2] = gmask[None, :]
    for i in range(2):
        cpk[:, CO_BQ + i] = bq_sa[i * 128:(i + 1) * 128]
        cpk[:, CO_BK + i] = bk_sa[i * 128:(i + 1) * 128]
        cpk[:, CO_BKX + i] = bk[hsl][i * 128:(i + 1) * 128]
    for i in range(DT):
        cpk[:, CO_BQX + i] = (bq * scale)[i * 128:(i + 1) * 128]
        cpk[:, CO_BO + i] = bo_sa[i * 128:(i + 1) * 128]
        cpk[:, CO_BOX + i] = bo_cx[i * 128:(i + 1) * 128]
        cpk[:, CO_B2 + i] = b2[i * 128:(i + 1) * 128]
    for i in range(FT):
        cpk[:, CO_B1 + i] = b1[i * 128:(i + 1) * 128]
    lng = np.concatenate([g_mha, g_crx, g_ffn])
    lnb = np.concatenate([bn_mha, bn_crx, bn_ffn])
    for i in range(3 * DT):
        cpk[:, CO_LNG + i] = lng[i * 128:(i + 1) * 128]
        cpk[:, CO_LNB + i] = lnb[i * 128:(i + 1) * 128]
    cpk[:, CO_BVA:CO_BVA + DCA] = bva_sa
    cpk[:, CO_BVXA:CO_BVXA + DCA] = bvx_c

    # causal mask tiles: msk[p, i*CW + j] = 1.0 if j - p >= 128*i else 0
    jj = np.arange(CW)[None, :]
    pp = np.arange(128)[:, None]
    mskpk = np.concatenate(
        [(jj - pp >= 128 * i).astype(f32) for i in range(4)], axis=1)

    return {
        "xpk": xpk,
        "mpk": mpk,
        "xspk": xspk,
        "wpk": wpk,
        "cpk": np.ascontiguousarray(cpk),
        "ones": np.full((128, 128), 1.0 / D, f32),
        "mskpk": np.ascontiguousarray(mskpk).astype(bf16),
        "wo": Wo_sa.astype(bf16),
        "wqx": (Wq * scale).astype(bf16),
        "wox": Wo_cx.astype(bf16),
        "w1": W1.astype(bf16),
        "w2": W2.astype(bf16),
    }


def kernel(**inputs):
    tgt = np.asarray(inputs["tgt"], np.float32)
    memory = np.asarray(inputs["memory"], np.float32)
    mask = np.asarray(inputs["tgt_mask"])
    S, M = tgt.shape[1], memory.shape[1]

    if mask.any():
        expect = np.triu(np.ones((S, S), bool), 1)
        if not np.array_equal(mask, expect):
            raise NotImplementedError("only causal or empty tgt_mask supported")
        causal = True
    else:
        causal = False

    nc = _get_nc(S, M, causal)
    args = {k: np.asarray(v, np.float32) for k, v in inputs.items()
            if k not in ("tgt", "memory", "tgt_mask")}
    in_maps = [_prep_inputs(c, S, M, tgt, memory, **args) for c in range(NCORES)]

    trace = bool(int(os.environ.get("BASS_KERNEL_TRACE", "0")))
    res = run_bass_kernel_spmd(nc, in_maps, list(range(NCORES)), trace=trace)
    kernel.last_raw_results = res.results
    if trace:
        kernel.last_exec_time_ns = res.exec_time_ns
        kernel.last_result = res
    out = np.empty((B, S, D), np.float32)
    for c in range(NCORES):
        r, b = c % G, c // G
        out[b, r * SW:(r + 1) * SW, :] = res.results[c]["outT"].T
    return out


# revision 47
# speedup vs baseline: 1.3776x; 1.0319x over previous
"""Trainium2 Bass kernel for nn_DecoderBlock (self-attn + cross-attn + FFN, post-LN).

Sharding (8 cores = 2 batch groups x 4 cores):
 - Attention is head-parallel within each group (4 heads/core, full S), which
   keeps the causal-mask instruction stream rank-uniform (SPMD).
 - Everything else (out-projections, LayerNorms, FFN) is sequence-parallel:
   each core owns a 512-token strip and uses the FULL Wo/W1/W2 weights, so
   there are no partial sums and no AllReduces.
 - Resharding transitions are 8-core AllToAlls, each split into two 1MB
   halves keyed by head-pair so the first half flies while the second
   head-pair is still being computed.  Receivers combine the two batch-group
   blocks with a per-core 0/1 mask (SPMD: rank variation lives in host data).
 - All matmul operands are bf16 (fp32 PSUM accumulation); the residual/LN
   stream is kept fp32.
 - Engine budget: scalar = exp/rsqrt only; vector = softmax denominators,
   masks, biases, LN arithmetic; gpsimd = collective triggers, A2A recv
   combines, LN squares/copies; sync = broadcast/spill DMAs.
"""

import os
import sys

sys.path.insert(0, "/opt/trn_rl_repo")

from contextlib import ExitStack

import ml_dtypes
import numpy as np

import concourse.bacc as bacc
import concourse.tile as tile
from concourse import mybir
from concourse.bass_utils import run_bass_kernel_spmd

F32R = mybir.dt.float32r
F32 = mybir.dt.float32
BF16 = mybir.dt.bfloat16
AF = mybir.ActivationFunctionType
ALU = mybir.AluOpType

B = 2
D = 1024
H = 16
HD = 64
FF = 4 * D
NCORES = 8
G = 4                  # cores per batch group
HL = H // G            # 4 local heads
DC = HL * HD           # 256 local q/k/v features
DCA = HL * (HD + 1)    # 260: V augmented with a ones column per head
SW = 512               # tokens per core strip
CW = 512               # free-dim chunk width
DT = D // 128          # 8 feature partition-tiles
FT = FF // 128         # 32 ffn partition-tiles
GROUP8 = [[0, 1, 2, 3, 4, 5, 6, 7]]

# bf16 weight pack: weight-major blocks, each [128, DT*cols] in (d, j) order
WO_Q = 0
WO_K = WO_Q + DT * DC
WO_V = WO_K + DT * DC
WO_KX = WO_V + DT * DCA
WO_VX = WO_KX + DT * DC
WCOLS = WO_VX + DT * DCA    # total pack columns

# f32 constant pack column offsets
CO_ONES = 0            # [128,128] ones/D
CO_GM = CO_ONES + 128  # gmask 2
CO_GMQ = CO_GM + 2     # (unused spare) 4
CO_BQ = CO_GMQ + 4     # bq 2
CO_BK = CO_BQ + 2      # bk 2
CO_BQX = CO_BK + 2     # bqx 8
CO_BKX = CO_BQX + DT   # bkx 2
CO_BO = CO_BKX + 2     # bo 8
CO_BOX = CO_BO + DT    # box 8
CO_B1 = CO_BOX + DT    # b1 32
CO_B2 = CO_B1 + FT     # b2 8
CO_LNG = CO_B2 + DT    # lng 24
CO_LNB = CO_LNG + 3 * DT  # lnb 24
CO_BVA = CO_LNB + 3 * DT  # bva 260 (partition-replicated)
CO_BVXA = CO_BVA + DCA    # bvxa 260
CCOLS = CO_BVXA + DCA

_nc_cache = {}


def _build(S, M, causal):
    nc = bacc.Bacc(None, target_bir_lowering=False, num_devices=NCORES)
    NCH = S // CW          # 4 query chunks (full S)
    ST = S // 128          # 16 key tiles (self)

    dp = nc.declare_dram_parameter
    xpk = dp("xpk", [128, DT * S], BF16, isOutput=False)
    mpk = dp("mpk", [128, DT * M], BF16, isOutput=False)
    xspk = dp("xspk", [128, DT * SW], F32R, isOutput=False)
    wpk = dp("wpk", [128, WCOLS], BF16, isOutput=False)
    cpk = dp("cpk", [128, CCOLS], F32, isOutput=False)
    ones = dp("ones", [128, 128], F32R, isOutput=False)
    mskpk = dp("mskpk", [128, 4 * CW], BF16, isOutput=False)
    wo = dp("wo", [D, D], BF16, isOutput=False)
    wqx = dp("wqx", [D, D], BF16, isOutput=False)
    wox = dp("wox", [D, D], BF16, isOutput=False)
    w1 = dp("w1", [D, FF], BF16, isOutput=False)
    w2 = dp("w2", [FF, D], BF16, isOutput=False)
    outT = dp("outT", [D, SW], F32R, isOutput=True)
    DEBUG = bool(int(os.environ.get("BASS_DEBUG_DUMP", "0")))
    if DEBUG:
        dbg_asa = dp("dbg_asa", [2 * 128, S], BF16, isOutput=True)
        dbg_astr = dp("dbg_astr", [D, SW], BF16, isOutput=True)
        dbg_z1 = dp("dbg_z1", [D, SW], F32R, isOutput=True)
        dbg_qx = dp("dbg_qx", [2 * 128, S], BF16, isOutput=True)
        dbg_kx = dp("dbg_kx", [2 * 128, M], BF16, isOutput=True)
        dbg_acx = dp("dbg_acx", [2 * 128, S], BF16, isOutput=True)
        dbg_ax = dp("dbg_ax", [D, SW], BF16, isOutput=True)
        dbg_z2 = dp("dbg_z2", [D, SW], F32R, isOutput=True)

    with tile.TileContext(nc) as tc, ExitStack() as st:
        ep = st.enter_context
        constp = ep(tc.tile_pool(name="const", bufs=1))
        dramp = ep(tc.tile_pool(name="dram", bufs=1, space="DRAM"))

        # gpsimd's queue blocks behind the collectives-prelude barrier and
        # behind any in-flight collective trigger, so anything that must
        # move while an AllToAll is airborne goes on the HWDGE queues.
        DMA_ENG = [nc.sync, nc.gpsimd, nc.scalar]   # collective-free phases
        LOADQ = [nc.sync, nc.scalar]                # always safe

        # ---- DRAM bounce buffers for collectives (1MB halves) ----
        t1in = [dramp.tile([D, SW], BF16, name=f"t1in{h}") for h in range(2)]
        t1out = [dramp.tile([D, SW], BF16, name=f"t1out{h}") for h in range(2)]
        q2in = [dramp.tile([D, SW], BF16, name=f"q2in{h}") for h in range(2)]
        q2out = [dramp.tile([D, SW], BF16, name=f"q2out{h}") for h in range(2)]
        t3in = [dramp.tile([D, SW], BF16, name=f"t3in{h}") for h in range(2)]
        t3out = [dramp.tile([D, SW], BF16, name=f"t3out{h}") for h in range(2)]

        # ---- packed constants: one DMA each ----
        cpk_t = constp.tile([128, CCOLS], F32, name="cpk_t")
        nc.scalar.dma_start(out=cpk_t[:], in_=cpk[:, :])
        msk_t = constp.tile([128, 4 * CW], BF16, name="msk_t")
        eps_t = constp.tile([128, 1], F32, name="eps_t")
        nc.vector.memset(eps_t[:], 1e-5)
        ones_t = constp.tile([128, 128], F32R, name="ones_t")
        gmask_t = cpk_t[:, CO_GM:CO_GM + 2]
        col = lambda off, i: cpk_t[:, off + i:off + i + 1]
        bq_t = [col(CO_BQ, i) for i in range(2)]
        bk_t = [col(CO_BK, i) for i in range(2)]
        bqx_t = [col(CO_BQX, i) for i in range(DT)]
        bkx_t = [col(CO_BKX, i) for i in range(2)]
        bo_t = [col(CO_BO, i) for i in range(DT)]
        box_t = [col(CO_BOX, i) for i in range(DT)]
        b1_t = [col(CO_B1, i) for i in range(FT)]
        b2_t = [col(CO_B2, i) for i in range(DT)]
        lng_t = [col(CO_LNG, i) for i in range(3 * DT)]
        lnb_t = [col(CO_LNB, i) for i in range(3 * DT)]
        bva_t = cpk_t[:, CO_BVA:CO_BVA + DCA]
        bvxa_t = cpk_t[:, CO_BVXA:CO_BVXA + DCA]

        # ================= helpers =================
        def project_qk(qt_pair, w_sl, b_tiles, xc, psum, tag):
            """q/k projection: 2x [128, S] packed tiles (2 heads each)."""
            for sc in range(NCH):
                sl = slice(sc * CW, (sc + 1) * CW)
                for t in range(2):
                    ps = psum.tile([128, CW], F32, name=f"{tag}ps", tag="qkps",
                                   bufs=2)
                    for d in range(DT):
                        nc.tensor.matmul(
                            out=ps[:],
                            lhsT=w_sl[d][:, t * 128:(t + 1) * 128],
                            rhs=xc[sc][d][:],
                            start=(d == 0), stop=(d == DT - 1),
                        )
                    nc.vector.tensor_scalar_add(
                        out=qt_pair[t][:, sl], in0=ps[:],
                        scalar1=b_tiles[t][:])

        def project_v(vpool, w_sl, bias_bc, xc, psum, n_tok, tag):
            """v projection: n_tok/128 tiles of [128 tok, DCA]."""
            vs = []
            kpc = CW // 128
            for s_t in range(n_tok // 128):
                sc, off = s_t // kpc, (s_t % kpc) * 128
                ps = psum.tile([128, DCA], F32, name=f"{tag}ps", tag="vps",
                               bufs=2)
                for d in range(DT):
                    nc.tensor.matmul(
                        out=ps[:],
                        lhsT=xc[sc][d][:, off:off + 128],
                        rhs=w_sl[d][:],
                        start=(d == 0), stop=(d == DT - 1))
                vt = vpool.tile([128, DCA], BF16, name=f"{tag}v", tag="vs",
                                bufs=2 * ST)
                nc.vector.tensor_tensor(out=vt[:], in0=ps[:], in1=bias_bc[:],
                                        op=ALU.add)
                vs.append(vt)
            return vs

        def attention(apool, ppool, dpool, q_tiles, k_tiles, v_tiles, n_keys,
                      use_mask, psc, ppv, tag, before_ti=None, after_ti=None):
            """Head-sharded attention, ti-OUTER loop; returns 2 packed
            [128, S] bf16 tiles (2 heads each).  before_ti(ti)/after_ti(ti)
            are hooks for assembling q just-in-time and spilling/triggering
            the output AllToAll half per head-pair."""
            a_packed = [apool.tile([128, S], BF16, name=f"{tag}{t}", tag="attn",
                                   bufs=2) for t in range(2)]
            kt_total = n_keys // 128
            kpc = CW // 128
            for ti in range(HL // 2):
                if before_ti is not None:
                    before_ti(ti)
                for qc in range(NCH):
                    sl = slice(qc * CW, (qc + 1) * CW)
                    kts = range(min(kt_total, kpc * (qc + 1)) if use_mask
                                else kt_total)
                    n_kt = len(kts)
                    pv_ps = [ppv.tile([65, CW], F32, name=f"{tag}pv{par}",
                                      tag="pvps", bufs=4) for par in range(2)]

                    def emit_scores(kt):
                        s_ps = psc.tile([128, 2 * CW], F32, name=f"{tag}s",
                                        tag="scps", bufs=2)
                        for par in range(2):
                            nc.tensor.matmul(
                                out=s_ps[:, par * CW:(par + 1) * CW],
                                lhsT=k_tiles[ti][par * 64:(par + 1) * 64,
                                                 kt * 128:(kt + 1) * 128],
                                rhs=q_tiles[ti][par * 64:(par + 1) * 64, sl],
                                start=True, stop=True,
                                tile_position=(64 * par, 0),
                            )
                        return s_ps

                    def emit_pv(s_ps, kt, i):
                        p_t = ppool.tile([128, 2 * CW], BF16, name=f"{tag}p",
                                         tag="p", bufs=4)
                        nc.scalar.activation(out=p_t[:], in_=s_ps[:],
                                             func=AF.Exp)
                        masked = use_mask and kt >= kpc * qc
                        if masked:
                            mi = kt - kpc * qc
                            p_m = ppool.tile([128, 2 * CW], BF16,
                                             name=f"{tag}pm", tag="p", bufs=4)
                            for par in range(2):
                                nc.vector.tensor_tensor(
                                    out=p_m[:, par * CW:(par + 1) * CW],
                                    in0=p_t[:, par * CW:(par + 1) * CW],
                                    in1=msk_t[:, mi * CW:(mi + 1) * CW],
                                    op=ALU.mult)
                            p_use = p_m
                        else:
                            p_use = p_t
                        for par in range(2):
                            h = 2 * ti + par
                            nc.tensor.matmul(
                                out=pv_ps[par][:],
                                lhsT=v_tiles[kt][:, h * 65:(h + 1) * 65],
                                rhs=p_use[:, par * CW:(par + 1) * CW],
                                start=(i == 0), stop=(i == n_kt - 1),
                            )

                    # one-step lookahead: scores of kt+1 are emitted before
                    # the exp-gated PV of kt so the PE never queues behind
                    # a semaphore wait on the scalar engine
                    prev = None
                    for i, kt in enumerate(kts):
                        s_ps = emit_scores(kt)
                        if prev is not None:
                            emit_pv(*prev)
                        prev = (s_ps, kt, i)
                    emit_pv(*prev)
                    for par in range(2):
                        # softmax denominator: copy row 64 out of PSUM,
                        # partition-broadcast, approx-reciprocal, multiply
                        # (reciprocal_approx_fast misbehaves on [1, N] APs
                        # at a partition offset, so it runs post-broadcast)
                        drc = dpool.tile([65, CW], F32, name=f"{tag}drc",
                                         tag="drc", bufs=2)
                        nc.vector.tensor_copy(out=drc[64:65, :],
                                              in_=pv_ps[par][64:65, :])
                        dn0 = dpool.tile([1, CW], F32, name=f"{tag}dn0",
                                         tag="dn0", bufs=2)
                        nc.sync.dma_start(out=dn0[0:1, :], in_=drc[64:65, :])
                        db = dpool.tile([64, CW], F32, name=f"{tag}db",
                                        tag="db", bufs=2)
                        nc.gpsimd.partition_broadcast(db[:], dn0[0:1, :])
                        dbr = dpool.tile([64, CW], F32, name=f"{tag}dbr",
                                         tag="dbr", bufs=2)
                        nc.vector.reciprocal_approx_fast(out=dbr[:], in_=db[:])
                        nc.vector.tensor_tensor(
                            out=a_packed[ti][par * 64:(par + 1) * 64, sl],
                            in0=pv_ps[par][0:64, :], in1=dbr[:], op=ALU.mult)
                if after_ti is not None:
                    after_ti(ti, a_packed[ti])
            return a_packed

        def a2a_spill_strip(a_tile, zin, j):
            """Spill dest-rank j's strip of one head-pair tile to DRAM."""
            LOADQ[j % len(LOADQ)].dma_start(
                out=zin[j * 128:(j + 1) * 128, :],
                in_=a_tile[:, (j % G) * SW:(j % G + 1) * SW])

        def a2a_trigger(zin, zout):
            nc.gpsimd.collective_compute(
                "AllToAll", ALU.bypass, replica_groups=GROUP8,
                ins=[zin.opt()], outs=[zout.opt()])

        def a2a_send(a_tile, zin, zout):
            for j in range(2 * G):
                a2a_spill_strip(a_tile, zin, j)
            a2a_trigger(zin, zout)

        def a2a_recv2(zouts, tmp_pool, a_pool, tag):
            """Strip assembly: ft feature-tile = head pair (ft//2 group-rank,
            ft%2 head-pair); masked 2-way combine of the batch-group blocks."""
            a_str = []
            for ft in range(DT):
                g, u = ft // 2, ft % 2
                z = zouts[u]
                top = tmp_pool.tile([128, SW], BF16, name=f"{tag}t", tag="atmp",
                                    bufs=8)
                bot = tmp_pool.tile([128, SW], BF16, name=f"{tag}b", tag="atmp",
                                    bufs=8)
                LOADQ[ft % len(LOADQ)].dma_start(
                    out=top[:], in_=z[g * 128:(g + 1) * 128, :])
                LOADQ[(ft + 1) % len(LOADQ)].dma_start(
                    out=bot[:], in_=z[(G + g) * 128:(G + g + 1) * 128, :])
                a = a_pool.tile([128, SW], BF16, name=f"{tag}a", tag="astr",
                                bufs=DT)
                nc.vector.tensor_scalar_mul(out=a[:], in0=top[:],
                                            scalar1=gmask_t[:, 0:1])
                nc.vector.scalar_tensor_tensor(
                    out=a[:], in0=bot[:], scalar=gmask_t[:, 1:2], in1=a[:],
                    op0=ALU.mult, op1=ALU.add)
                a_str.append(a)
            return a_str

        def out_project_strip(wo_tiles, b_tiles, a_str, z32, psum, tag):
            """z32[d] = wo.T @ a_str + bias + z32  (residual add in place)."""
            for d in range(DT):
                ps = psum.tile([128, CW], F32, name=f"{tag}ps", tag="ops",
                               bufs=2)
                for kt in range(DT):
                    nc.tensor.matmul(
                        out=ps[:], lhsT=wo_tiles[kt][:, d * 128:(d + 1) * 128],
                        rhs=a_str[kt][:],
                        start=(kt == 0), stop=(kt == DT - 1))
                nc.vector.scalar_tensor_tensor(
                    out=z32[d][:], in0=ps[:], scalar=b_tiles[d][:],
                    in1=z32[d][:], op0=ALU.add, op1=ALU.add)

        def layer_norm_strip(z32, z16, ln_idx, psum, sqpool, tmppool, tag):
            """Post-LN on the [D, SW] fp32 strip; writes bf16 copy z16."""
            mps = psum.tile([128, CW], F32, name=f"{tag}m", tag="lnps", bufs=2)
            for d in range(DT):
                nc.tensor.matmul(out=mps[:], lhsT=ones_t[:],
                                 rhs=z32[d][:],
                                 start=(d == 0), stop=(d == DT - 1))
            mu = tmppool.tile([128, CW], F32, name=f"{tag}mu", tag="mu", bufs=1)
            nc.vector.tensor_copy(out=mu[:], in_=mps[:])
            qps = psum.tile([128, CW], F32, name=f"{tag}q", tag="lnps", bufs=2)
            for d in range(DT):
                sq = sqpool.tile([128, CW], F32R, name=f"{tag}sq", tag="sq",
                                 bufs=2)
                nc.gpsimd.tensor_tensor(out=sq[:], in0=z32[d][:],
                                        in1=z32[d][:], op=ALU.mult)
                nc.tensor.matmul(out=qps[:], lhsT=ones_t[:], rhs=sq[:],
                                 start=(d == 0), stop=(d == DT - 1))
            var = tmppool.tile([128, CW], F32, name=f"{tag}v", tag="tv", bufs=1)
            nc.vector.tensor_tensor(out=var[:], in0=mu[:], in1=mu[:],
                                    op=ALU.mult)
            nc.vector.tensor_tensor(out=var[:], in0=qps[:], in1=var[:],
                                    op=ALU.subtract)
            std = tmppool.tile([128, CW], F32, name=f"{tag}sd", tag="std",
                               bufs=1)
            nc.scalar.activation(out=std[:], in_=var[:], func=AF.Sqrt,
                                 bias=eps_t[:], scale=1.0)
            rstd = tmppool.tile([128, CW], F32, name=f"{tag}r", tag="rstd",
                                bufs=1)
            nc.vector.reciprocal_approx_fast(out=rstd[:], in_=std[:])
            for d in range(DT):
                xm = tmppool.tile([128, CW], F32, name=f"{tag}x", tag="xm",
                                  bufs=2)
                nc.gpsimd.tensor_tensor(out=xm[:], in0=z32[d][:], in1=mu[:],
                                        op=ALU.subtract)
                nc.vector.tensor_tensor(out=xm[:], in0=xm[:], in1=rstd[:],
                                        op=ALU.mult)
                nc.vector.tensor_scalar(
                    out=z32[d][:], in0=xm[:],
                    scalar1=lng_t[ln_idx * DT + d][:],
                    scalar2=lnb_t[ln_idx * DT + d][:],
                    op0=ALU.mult, op1=ALU.add)
                if z16 is not None:
                    nc.scalar.copy(out=z16[d][:], in_=z32[d][:])

        # ================= pipeline =================
        zp = ep(tc.tile_pool(name="zp", bufs=1))
        z16p = ep(tc.tile_pool(name="z16p", bufs=DT))
        wop = ep(tc.tile_pool(name="wop", bufs=DT))
        apool = ep(tc.tile_pool(name="apool", bufs=2))
        atmpp = ep(tc.tile_pool(name="atmpp", bufs=8))
        astrp = ep(tc.tile_pool(name="astrp", bufs=DT))

        attn_stack = ExitStack()
        ap2 = attn_stack.enter_context
        mpool = ap2(tc.tile_pool(name="memp", bufs=1))
        qkp = ap2(tc.tile_pool(name="qk", bufs=8))
        vp = ap2(tc.tile_pool(name="vp", bufs=2 * ST))
        wqkp = ap2(tc.tile_pool(name="wqk", bufs=1))

        # ---- P0/P1: loads + self QKV ----
        # x pack is CHUNK-major: one 1MB DMA per 512-token chunk so the
        # first projection starts after ~1MB instead of the full 4MB
        with tc.tile_pool(name="xp", bufs=1) as xpool:
            x_t = xpool.tile([128, DT * S], BF16, name="xt", tag="x", bufs=1)
            w_t = wqkp.tile([128, WCOLS], BF16, name="wt", tag="w", bufs=1)
            wsl = lambda base, cols: [
                w_t[:, base + d * cols:base + (d + 1) * cols]
                for d in range(DT)]
            wq_t = wsl(WO_Q, DC)
            wk_t = wsl(WO_K, DC)
            wv_t = wsl(WO_V, DCA)
            wkx_t = wsl(WO_KX, DC)
            wvx_t = wsl(WO_VX, DCA)
            CHW = DT * CW                 # pack columns per x chunk
            xc = [[x_t[:, sc * CHW + d * CW:sc * CHW + (d + 1) * CW]
                   for d in range(DT)] for sc in range(NCH)]

            # interleaved startup loads: scalar carries weights, sync the
            # activations; chunk 0 + wq land first
            nc.sync.dma_start(out=x_t[:, 0:CHW], in_=xpk[:, 0:CHW])
            nc.scalar.dma_start(out=w_t[:, WO_Q:WO_K], in_=wpk[:, WO_Q:WO_K])
            nc.scalar.dma_start(out=w_t[:, WO_K:WO_V], in_=wpk[:, WO_K:WO_V])
            nc.sync.dma_start(out=x_t[:, CHW:2 * CHW], in_=xpk[:, CHW:2 * CHW])
            nc.scalar.dma_start(out=w_t[:, WO_V:WO_KX], in_=wpk[:, WO_V:WO_KX])
            nc.sync.dma_start(out=x_t[:, 2 * CHW:3 * CHW],
                              in_=xpk[:, 2 * CHW:3 * CHW])
            nc.scalar.dma_start(out=x_t[:, 3 * CHW:4 * CHW],
                                in_=xpk[:, 3 * CHW:4 * CHW])
            nc.scalar.dma_start(out=w_t[:, WO_KX:], in_=wpk[:, WO_KX:])
            nc.scalar.dma_start(out=msk_t[:], in_=mskpk[:, :])
            nc.scalar.dma_start(out=ones_t[:], in_=ones[:, :])

            m_t = mpool.tile([128, DT * M], BF16, name="mt", tag="m", bufs=1)
            mhalf = DT * M // 2
            nc.scalar.dma_start(out=m_t[:, :mhalf], in_=mpk[:, :mhalf])
            nc.sync.dma_start(out=m_t[:, mhalf:], in_=mpk[:, mhalf:])
            m_fm = [m_t[:, d * M:(d + 1) * M] for d in range(DT)]

            z_t = zp.tile([128, DT * SW], F32R, name="z32", tag="z32", bufs=1)
            nc.sync.dma_start(out=z_t[:], in_=xspk[:, :])
            z32 = [z_t[:, d * SW:(d + 1) * SW] for d in range(DT)]
            z16 = [z16p.tile([128, SW], BF16, name="z16", tag="z16", bufs=DT)
                   for _ in range(DT)]

            q_s = [qkp.tile([128, S], BF16, name="qs", tag="qk", bufs=8)
                   for _ in range(2)]
            k_s = [qkp.tile([128, S], BF16, name="ks", tag="qk", bufs=8)
                   for _ in range(2)]
            with tc.tile_pool(name="ps1", bufs=2, space="PSUM") as ps1, \
                 tc.tile_pool(name="ps1v", bufs=2, space="PSUM") as ps1v:
                project_qk(q_s, wq_t, bq_t, xc, ps1, "qs")
                project_qk(k_s, wk_t, bk_t, xc, ps1, "ks")
                v_s = project_v(vp, wv_t, bva_t, xc, ps1v, S, "vs")

        # x freed here; z32 holds the residual strip.

        # ---- P2/P3: self attention with mid-flight A2A halves ----
        kx = [qkp.tile([128, M], BF16, name="kx", tag="qk", bufs=8)
              for _ in range(2)]
        vx = []

        def ckv_kx(psx, lo, hi):
            for sc in range(lo, hi):
                sl = slice(sc * CW, (sc + 1) * CW)
                for t in range(2):
                    ps = psx.tile([128, CW], F32, name="kxps", tag="kxps",
                                  bufs=2)
                    for d in range(DT):
                        nc.tensor.matmul(
                            out=ps[:],
                            lhsT=wkx_t[d][:, t * 128:(t + 1) * 128],
                            rhs=m_fm[d][:, sl],
                            start=(d == 0), stop=(d == DT - 1))
                    nc.vector.tensor_scalar_add(out=kx[t][:, sl], in0=ps[:],
                                                scalar1=bkx_t[t][:])

        def ckv_vx(psxv, lo, hi):
            for s_t in range(lo, hi):
                ps = psxv.tile([128, DCA], F32, name="vxps", tag="vxps",
                               bufs=2)
                for d in range(DT):
                    nc.tensor.matmul(
                        out=ps[:],
                        lhsT=m_fm[d][:, s_t * 128:(s_t + 1) * 128],
                        rhs=wvx_t[d][:],
                        start=(d == 0), stop=(d == DT - 1))
                vt = vp.tile([128, DCA], BF16, name="vx", tag="vs",
                             bufs=2 * ST)
                nc.vector.tensor_tensor(out=vt[:], in0=ps[:],
                                        in1=bvxa_t[:], op=ALU.add)
                vx.append(vt)

        def sa_after_ti(ti, a_tile):
            if DEBUG:
                nc.sync.dma_start(out=dbg_asa[ti * 128:(ti + 1) * 128, :],
                                  in_=a_tile[:])
            for j in range(2 * G):
                a2a_spill_strip(a_tile, t1in[ti], j)
            a2a_trigger(t1in[ti], t1out[ti])

        with tc.tile_pool(name="pp1", bufs=4) as pp1, \
             tc.tile_pool(name="dn1", bufs=6) as dn1, \
             tc.tile_pool(name="ps2s", bufs=2, space="PSUM") as ps2s, \
             tc.tile_pool(name="ps2v", bufs=4, space="PSUM") as ps2v:
            attention(apool, pp1, dn1, q_s, k_s, v_s, S, causal,
                      ps2s, ps2v, "sa", after_ti=sa_after_ti)

        # ---- P4/P5: cross-K fills the A2A#1b window; recv; oproj; LN1 ----
        with tc.tile_pool(name="psx", bufs=2, space="PSUM") as psx:
            ckv_kx(psx, 0, NCH)
        wo_t = []
        for k in range(DT):
            t = wop.tile([128, D], BF16, name="wot", tag="wot", bufs=DT)
            LOADQ[k % len(LOADQ)].dma_start(
                out=t[:], in_=wo[k * 128:(k + 1) * 128, :])
            wo_t.append(t)
        with tc.tile_pool(name="ps3", bufs=2, space="PSUM") as ps3, \
             tc.tile_pool(name="sqA", bufs=2) as sqA, \
             tc.tile_pool(name="tmA", bufs=4) as tmA:
            # the wait gate keeps the scheduler from hoisting these
            # collective-gated loads into the attention-tail engine queues
            with tc.tile_wait_until(1.0):
                a_str = a2a_recv2(t1out, atmpp, astrp, "a1")
            if DEBUG:
                for ft in range(DT):
                    nc.sync.dma_start(
                        out=dbg_astr[ft * 128:(ft + 1) * 128, :],
                        in_=a_str[ft][:])
            out_project_strip(wo_t, bo_t, a_str, z32, ps3, "o1")
            layer_norm_strip(z32, z16, 0, ps3, sqA, tmA, "l1")
            if DEBUG:
                for d in range(DT):
                    nc.sync.dma_start(
                        out=dbg_z1[d * 128:(d + 1) * 128, :], in_=z32[d][:])

        # cross-Q: project full q on my strip, then AllToAll halves to
        # head-shard it; remaining cross-V fills the first half's window
        with tc.tile_pool(name="wqxp", bufs=DT) as wqxp, \
             tc.tile_pool(name="qfp", bufs=DT) as qfp, \
             tc.tile_pool(name="ps5q", bufs=2, space="PSUM") as ps5q:
            wqxf_t = []
            for k in range(DT):
                t = wqxp.tile([128, D], BF16, name="wqxf", tag="wqxf", bufs=DT)
                LOADQ[k % len(LOADQ)].dma_start(
                    out=t[:], in_=wqx[k * 128:(k + 1) * 128, :])
                wqxf_t.append(t)
            qf = [None] * DT

            def qf_pt(pt):
                ps = ps5q.tile([128, CW], F32, name="qfps", tag="qf", bufs=2)
                for d in range(DT):
                    nc.tensor.matmul(
                        out=ps[:], lhsT=wqxf_t[d][:, pt * 128:(pt + 1) * 128],
                        rhs=z16[d][:],
                        start=(d == 0), stop=(d == DT - 1))
                t = qfp.tile([128, SW], BF16, name="qf", tag="qf", bufs=DT)
                nc.vector.tensor_scalar_add(out=t[:], in0=ps[:],
                                            scalar1=bqx_t[pt][:])
                qf[pt] = t

            # even head-pair tiles first so the first q-AllToAll half can
            # launch while the odd tiles and cross-V still compute
            for u in range(2):
                for pt in range(u, DT, 2):
                    qf_pt(pt)
                for j in range(2 * G):
                    LOADQ[j % len(LOADQ)].dma_start(
                        out=q2in[u][j * 128:(j + 1) * 128, :],
                        in_=qf[2 * (j % G) + u][:])
                a2a_trigger(q2in[u], q2out[u])
            # cross-V fills the q-A2A windows
            with tc.tile_pool(name="psxv", bufs=2, space="PSUM") as psxv:
                ckv_vx(psxv, 0, M // 128)

        # ---- P6: cross attention; q halves assembled just-in-time ----
        q_x = [qkp.tile([128, S], BF16, name="qx", tag="qk", bufs=8)
               for _ in range(2)]

        def cx_before_ti(u):
            st_qx = ExitStack()
            st_qx.enter_context(tc.tile_wait_until(1.5 + 0.5 * u))
            for c in range(G):
                top = atmpp.tile([128, SW], BF16, name="qxt", tag="atmp",
                                 bufs=8)
                bot = atmpp.tile([128, SW], BF16, name="qxb", tag="atmp",
                                 bufs=8)
                LOADQ[c % len(LOADQ)].dma_start(
                    out=top[:], in_=q2out[u][c * 128:(c + 1) * 128, :])
                LOADQ[(c + 1) % len(LOADQ)].dma_start(
                    out=bot[:], in_=q2out[u][(G + c) * 128:(G + c + 1) * 128, :])
                slc = slice(c * SW, (c + 1) * SW)
                nc.vector.tensor_scalar_mul(out=q_x[u][:, slc], in0=top[:],
                                            scalar1=gmask_t[:, 0:1])
                nc.vector.scalar_tensor_tensor(
                    out=q_x[u][:, slc], in0=bot[:], scalar=gmask_t[:, 1:2],
                    in1=q_x[u][:, slc], op0=ALU.mult, op1=ALU.add)
            st_qx.close()

        def cx_after_ti(ti, a_tile):
            if DEBUG:
                nc.sync.dma_start(out=dbg_acx[ti * 128:(ti + 1) * 128, :],
                                  in_=a_tile[:])
                nc.sync.dma_start(out=dbg_qx[ti * 128:(ti + 1) * 128, :],
                                  in_=q_x[ti][:])
                nc.sync.dma_start(out=dbg_kx[ti * 128:(ti + 1) * 128, :],
                                  in_=kx[ti][:])
            for j in range(2 * G):
                a2a_spill_strip(a_tile, t3in[ti], j)
            a2a_trigger(t3in[ti], t3out[ti])

        with tc.tile_pool(name="pp2", bufs=4) as pp2, \
             tc.tile_pool(name="dn2", bufs=6) as dn2, \
             tc.tile_pool(name="ps6s", bufs=2, space="PSUM") as ps6s, \
             tc.tile_pool(name="ps6v", bufs=4, space="PSUM") as ps6v:
            a_cx = attention(apool, pp2, dn2, q_x, kx, vx, M, False,
                             ps6s, ps6v, "cx", before_ti=cx_before_ti,
                             after_ti=cx_after_ti)

        attn_stack.close()  # frees mem, q/k/v, weights for qkv

        # ---- P7/P8: A2A#3b window (warm-keeper matmuls), oproj, LN2 ----
        wox_t = []
        for k in range(DT):
            t = wop.tile([128, D], BF16, name="wot", tag="wot", bufs=DT)
            LOADQ[k % len(LOADQ)].dma_start(
                out=t[:], in_=wox[k * 128:(k + 1) * 128, :])
            wox_t.append(t)
        # keep the PE's HAM clock warm while A2A#3b drains: ~60 throwaway
        # matmuls reading the just-finished attention tile (the data dep
        # pins them to the gap; they never outrun the real work)
        with tc.tile_pool(name="wk", bufs=1, space="PSUM") as wkp:
            wps = wkp.tile([128, CW], F32, name="wkps", tag="wkps", bufs=1)
            for i in range(60):
                nc.tensor.matmul(out=wps[:], lhsT=wox_t[i % DT][:, 0:128],
                                 rhs=a_cx[1][:, (NCH - 1) * CW:NCH * CW],
                                 start=True, stop=True)
        with tc.tile_pool(name="ps7", bufs=2, space="PSUM") as ps7, \
             tc.tile_pool(name="sqB", bufs=2) as sqB, \
             tc.tile_pool(name="tmB", bufs=4) as tmB:
            with tc.tile_wait_until(3.0):
                ax_str = a2a_recv2(t3out, atmpp, astrp, "a3")
            if DEBUG:
                for ft in range(DT):
                    nc.sync.dma_start(
                        out=dbg_ax[ft * 128:(ft + 1) * 128, :],
                        in_=ax_str[ft][:])
            out_project_strip(wox_t, box_t, ax_str, z32, ps7, "o2")
            layer_norm_strip(z32, z16, 1, ps7, sqB, tmB, "l2")
            if DEBUG:
                for d in range(DT):
                    nc.sync.dma_start(
                        out=dbg_z2[d * 128:(d + 1) * 128, :], in_=z32[d][:])

        # ---- P9: FFN + LN3 + output ----
        with tc.tile_pool(name="hp", bufs=FT) as hp, \
             tc.tile_pool(name="w1sp", bufs=6) as w1sp, \
             tc.tile_pool(name="w2p", bufs=6) as w2p, \
             tc.tile_pool(name="sqC", bufs=2) as sqC, \
             tc.tile_pool(name="tmC", bufs=4) as tmC:
            h_t = [None] * FT
            # FF1 in f-blocks of 8: stream w1 row-tiles [128(d), 1024(f-blk)]
            with tc.tile_pool(name="ps9a", bufs=8, space="PSUM") as ps9a:
                for fb in range(FT // 8):
                    f1ps = [ps9a.tile([128, CW], F32, name=f"f1ps{i}",
                                      tag=f"f1ps{i}", bufs=1)
                            for i in range(8)]
                    for d in range(DT):
                        w1t = w1sp.tile([128, 1024], BF16, name="w1t", tag="w1",
                                        bufs=6)
                        DMA_ENG[d % len(DMA_ENG)].dma_start(
                            out=w1t[:],
                            in_=w1[d * 128:(d + 1) * 128,
                                   fb * 1024:(fb + 1) * 1024])
                        for i in range(8):
                            nc.tensor.matmul(
                                out=f1ps[i][:],
                                lhsT=w1t[:, i * 128:(i + 1) * 128],
                                rhs=z16[d][:],
                                start=(d == 0), stop=(d == DT - 1))
                    for i in range(8):
                        f = fb * 8 + i
                        ht = hp.tile([128, CW], BF16, name="ht", tag="h",
                                     bufs=FT)
                        nc.scalar.activation(out=ht[:], in_=f1ps[i][:],
                                             func=AF.Relu, bias=b1_t[f][:],
                                             scale=1.0)
                        h_t[f] = ht
            # FF2: f-outer accumulation into 8 concurrent psum banks
            with tc.tile_pool(name="ps9b", bufs=8, space="PSUM") as ps9b:
                f2ps = [ps9b.tile([128, CW], F32, name=f"f2ps{d}",
                                  tag=f"f2ps{d}", bufs=1) for d in range(DT)]
                for f in range(FT):
                    w2t = w2p.tile([128, D], BF16, name="w2t", tag="w2", bufs=6)
                    DMA_ENG[f % len(DMA_ENG)].dma_start(
                        out=w2t[:], in_=w2[f * 128:(f + 1) * 128, :])
                    for d in range(DT):
                        nc.tensor.matmul(
                            out=f2ps[d][:], lhsT=w2t[:, d * 128:(d + 1) * 128],
                            rhs=h_t[f][:],
                            start=(f == 0), stop=(f == FT - 1))
                for d in range(DT):
                    nc.vector.scalar_tensor_tensor(
                        out=z32[d][:], in0=f2ps[d][:], scalar=b2_t[d][:],
                        in1=z32[d][:], op0=ALU.add, op1=ALU.add)
            with tc.tile_pool(name="ps9c", bufs=2, space="PSUM") as ps9c:
                layer_norm_strip(z32, None, 2, ps9c, sqC, tmC, "l3")
            for d in range(DT):
                DMA_ENG[d % len(DMA_ENG)].dma_start(
                    out=outT[d * 128:(d + 1) * 128, :], in_=z32[d][:])

    nc.finalize()
    return nc


def _get_nc(S, M, causal):
    key = (S, M, causal)
    if key not in _nc_cache:
        _nc_cache[key] = _build(S, M, causal)
    return _nc_cache[key]


def _prep_inputs(c, S, M, tgt, memory, Wqkv, bqkv, Wo_sa, bo_sa, Wq, bq, Wk, bk,
                 Wv, bv, Wo_cx, bo_cx, W1, b1, W2, b2, g_mha, bn_mha, g_crx,
                 bn_crx, g_ffn, bn_ffn):
    r, b = c % G, c // G
    hsl = slice(r * DC, (r + 1) * DC)
    f32 = np.float32
    bf16 = ml_dtypes.bfloat16

    def aug_v(wv_c, bv_c):
        wva = np.zeros((D, DCA), f32)
        bva = np.zeros((1, DCA), f32)
        for h in range(HL):
            wva[:, h * 65:h * 65 + 64] = wv_c[:, h * 64:(h + 1) * 64]
            bva[0, h * 65:h * 65 + 64] = bv_c[h * 64:(h + 1) * 64]
            bva[0, h * 65 + 64] = 1.0
        return wva, bva

    scale = np.float32(1.0 / np.sqrt(HD))
    wqkv_h = Wqkv.reshape(D, H, 3 * HD)
    bqkv_h = bqkv.reshape(H, 3 * HD)
    gh = slice(r * HL, (r + 1) * HL)
    wq_sa = wqkv_h[:, gh, 0:HD].reshape(D, DC) * scale
    wk_sa = wqkv_h[:, gh, HD:2 * HD].reshape(D, DC)
    wv_sa = wqkv_h[:, gh, 2 * HD:3 * HD].reshape(D, DC)
    bq_sa = bqkv_h[gh, 0:HD].reshape(DC) * scale
    bk_sa = bqkv_h[gh, HD:2 * HD].reshape(DC)
    bv_sa = bqkv_h[gh, 2 * HD:3 * HD].reshape(DC)
    wva_sa, bva_sa = aug_v(wv_sa, bv_sa)
    wvx_c, bvx_c = aug_v(Wv[:, hsl], bv[hsl])
    xT_full = np.ascontiguousarray(tgt[b].T)  # [D, S]
    g0 = np.float32(1.0 if b == 0 else 0.0)
    gmask = np.array([g0, 1.0 - g0], f32)

    # packed x: xpk[p, d*S + j] = xT[d*128 + p, j]
    def pack_rows(a, cols):
        # a: [D, cols] -> [128, DT*cols]
        return np.ascontiguousarray(
            a.reshape(DT, 128, cols).transpose(1, 0, 2).reshape(128, DT * cols))

    # x pack is chunk-major: xpk[p, sc*DT*CW + d*CW + j] = xT[d*128+p, sc*CW+j]
    NCH, CW = S // 512, 512
    xpk = np.ascontiguousarray(
        xT_full.astype(bf16).reshape(DT, 128, NCH, CW)
        .transpose(1, 2, 0, 3).reshape(128, S * DT))
    mpk = pack_rows(np.ascontiguousarray(memory[b].T).astype(bf16), M)
    xspk = pack_rows(
        np.ascontiguousarray(xT_full[:, r * SW:(r + 1) * SW], f32), SW)

    # packed weights, weight-major: each block [128, DT*cols] in (d, j) order
    wpk = np.hstack([
        pack_rows(wq_sa.astype(bf16), DC),
        pack_rows(np.ascontiguousarray(wk_sa).astype(bf16), DC),
        pack_rows(wva_sa.astype(bf16), DCA),
        pack_rows(np.ascontiguousarray(Wk[:, hsl]).astype(bf16), DC),
        pack_rows(wvx_c.astype(bf16), DCA),
    ])

    # packed f32 constants
    cpk = np.zeros((128, CCOLS), f32)
    cpk[:, CO_ONES:CO_ONES + 128] = 1.0 / D
    cpk[:, CO_GM:CO_GM + 2] = gmask[None, :]
    for i in range(2):
        cpk[:, CO_BQ + i] = bq_sa[i * 128:(i + 1) * 128]
        cpk[:, CO_BK + i] = bk_sa[i * 128:(i + 1) * 128]
        cpk[:, CO_BKX + i] = bk[hsl][i * 128:(i + 1) * 128]
    for i in range(DT):
        cpk[:, CO_BQX + i] = (bq * scale)[i * 128:(i + 1) * 128]
        cpk[:, CO_BO + i] = bo_sa[i * 128:(i + 1) * 128]
        cpk[:, CO_BOX + i] = bo_cx[i * 128:(i + 1) * 128]
        cpk[:, CO_B2 + i] = b2[i * 128:(i + 1) * 128]
    for i in range(FT):
        cpk[:, CO_B1 + i] = b1[i * 128:(i + 1) * 128]
    lng = np.concatenate([g_mha, g_crx, g_ffn])
    lnb = np.concatenate([bn_mha, bn_crx, bn_ffn])
    for i in range(3 * DT):
        cpk[:, CO_LNG + i] = lng[i * 128:(i + 1) * 128]
        cpk[:, CO_LNB + i] = lnb[i * 128:(i + 1) * 128]
    cpk[:, CO_BVA:CO_BVA + DCA] = bva_sa
    cpk[:, CO_BVXA:CO_BVXA + DCA] = bvx_c

    # causal mask tiles: msk[p, i*CW + j] = 1.0 if j - p >= 128*i else 0
    jj = np.arange(CW)[None, :]
    pp = np.arange(128)[:, None]
    mskpk = np.concatenate(
        [(jj - pp >= 128 * i).astype(f32) for i in range(4)], axis=1)

    return {
        "xpk": xpk,
        "mpk": mpk,
        "xspk": xspk,
        "wpk": wpk,
        "cpk": np.ascontiguousarray(cpk),
        "ones": np.full((128, 128), 1.0 / D, f32),
        "mskpk": np.ascontiguousarray(mskpk).astype(bf16),
        "wo": Wo_sa.astype(bf16),
        "wqx": (Wq * scale).astype(bf16),
        "wox": Wo_cx.astype(bf16),
        "w1": W1.astype(bf16),
        "w2": W2.astype(bf16),
    }


def kernel(**inputs):
    tgt = np.asarray(inputs["tgt"], np.float32)
    memory = np.asarray(inputs["memory"], np.float32)
    mask = np.asarray(inputs["tgt_mask"])
    S, M = tgt.shape[1], memory.shape[1]

    if mask.any():
        expect = np.triu(np.ones((S, S), bool), 1)
        if not np.array_equal(mask, expect):
            raise NotImplementedError("only causal or empty tgt_mask supported")
        causal = True
    else:
        causal = False

    nc = _get_nc(S, M, causal)
    args = {k: np.asarray(v, np.float32) for k, v in inputs.items()
            if k not in ("tgt", "memory", "tgt_mask")}
    in_maps = [_prep_inputs(c, S, M, tgt, memory, **args) for c in range(NCORES)]

    trace = bool(int(os.environ.get("BASS_KERNEL_TRACE", "0")))
    res = run_bass_kernel_spmd(nc, in_maps, list(range(NCORES)), trace=trace)
    kernel.last_raw_results = res.results
    if trace:
        kernel.last_exec_time_ns = res.exec_time_ns
        kernel.last_result = res
    out = np.empty((B, S, D), np.float32)
    for c in range(NCORES):
        r, b = c % G, c // G
        out[b, r * SW:(r + 1) * SW, :] = res.results[c]["outT"].T
    return out
